# revision 1
# baseline (speedup 1.0000x reference)
"""Trainium2 Bass kernel for nn_ContrastiveEncoderMOE.

Strategy: data-parallel over batch (4 batches per core, 8 cores, no
collectives). Two device launches inside kernel():
  A) router: conv -> GroupNorm -> GELU -> GAP -> MLP(+LN) -> concat demo
     embedding -> gate logits  (per-core output: (8,4) logits)
  host: softmax + top-2 + renormalize on (32,8); gather the 2 selected
     experts' conv weights per batch (control-plane only).
  B) shared conv + 2 selected expert convs per batch, GroupNorm+GELU,
     weighted combine, full (4,128,2048) output per core.

Convs are 5 shifted matmuls (contraction C=16(+1 bias row)) accumulated in
PSUM, batches packed into the 4 PE row-groups. GroupNorm stats via bn_stats
on PSUM with a second conv pass for the normalize+GELU read (recompute is
cheaper than spilling h). Matmul inputs are float32r (full-rate on PE).
"""

import numpy as np

B, C, T = 32, 16, 2048
E, CO, K = 8, 128, 5
HID, CTX, DIN, DEMB = 128, 64, 8, 16
GROUPS = 8
NCORES = 8
BPC = B // NCORES  # batches per core
TPAD = T + K - 1  # 2052
EPS = 1e-5
GSZ = CO // GROUPS  # 16 channels per group
NTT = T // 512  # 4 T-tiles of 512
NCH = T // 1024  # 2 chunks of 1024 (kernel B)

_built = {}


def _split_multiwait(nc, max_waits=1):
    # The pinned walrus rejects >1 sync-wait on one instruction
    # ("Too many sync wait commands"); hoist excess waits onto
    # same-engine NOPs inserted just before.
    from concourse import mybir

    for f in nc.m.functions:
        for blk in f.blocks:
            out = []
            for inst in blk.instructions:
                si = getattr(inst, "sync_info", None)
                if si is not None and si.on_wait and len(si.on_wait) > max_waits:
                    waits = list(si.on_wait)
                    cnt = 0
                    while len(waits) > max_waits:
                        chunk, waits = waits[:max_waits], waits[max_waits:]
                        nop = mybir.InstNoOp(
                            name=f"{inst.name}-mw{cnt}",
                            engine=inst.engine,
                            bass_nofuse=True,
                            sync_info=mybir.SyncInfo(on_wait=chunk, on_update=[]),
                        )
                        out.append(nop)
                        cnt += 1
                    inst.sync_info = mybir.SyncInfo(
                        on_wait=waits, on_update=list(si.on_update)
                    )
                out.append(inst)
            blk.instructions[:] = out
    return nc


def _build_a():
    import concourse.bass as bass
    import concourse.tile as tile
    from concourse import mybir

    f32 = mybir.dt.float32
    f32r = mybir.dt.float32r
    FT = mybir.ActivationFunctionType
    AL = mybir.AluOpType
    AX = mybir.AxisListType

    nc = bass.Bass()
    xin = nc.dram_tensor("xin", [128, TPAD], f32r, kind="ExternalInput")
    rwt = nc.dram_tensor("rwt", [128, K * 128], f32r, kind="ExternalInput")
    gind = nc.dram_tensor("gind", [128, GROUPS], f32, kind="ExternalInput")
    gindT = nc.dram_tensor("gindT", [GROUPS, 128], f32, kind="ExternalInput")
    rgb = nc.dram_tensor("rgb", [128, 2], f32, kind="ExternalInput")  # rg, rb
    m1wt = nc.dram_tensor("m1wt", [128, HID], f32, kind="ExternalInput")
    lnp = nc.dram_tensor("lnp", [128, 3], f32, kind="ExternalInput")  # b1,lng,lnb
    m2wt = nc.dram_tensor("m2wt", [128, CTX], f32, kind="ExternalInput")
    b2 = nc.dram_tensor("b2", [CTX, 1], f32, kind="ExternalInput")
    demoT = nc.dram_tensor("demoT", [DIN, BPC], f32, kind="ExternalInput")
    d1wt = nc.dram_tensor("d1wt", [DIN, 2 * DEMB], f32, kind="ExternalInput")
    dlnp = nc.dram_tensor("dlnp", [2 * DEMB, 3], f32, kind="ExternalInput")
    d2wt = nc.dram_tensor("d2wt", [2 * DEMB, DEMB], f32, kind="ExternalInput")
    db2 = nc.dram_tensor("db2", [DEMB, 1], f32, kind="ExternalInput")
    gwt = nc.dram_tensor("gwt", [CTX + DEMB, E], f32, kind="ExternalInput")
    gbi = nc.dram_tensor("gbi", [E, 1], f32, kind="ExternalInput")
    logout = nc.dram_tensor("logitsT", [E, BPC], f32, kind="ExternalOutput")

    with tile.TileContext(nc) as tc:
        with (
            tc.tile_pool(name="const", bufs=1) as cst,
            tc.tile_pool(name="stats", bufs=1) as stp,
            tc.tile_pool(name="work", bufs=1) as wrk,
            tc.tile_pool(name="scratch", bufs=4) as scr,
            tc.tile_pool(name="cps", bufs=6, space="PSUM") as cps,
            tc.tile_pool(name="sps", bufs=2, space="PSUM") as sps,
        ):
            dma = nc.gpsimd.dma_start
            # ---- load constants
            x_t = cst.tile([128, TPAD], f32r)
            dma(out=x_t, in_=xin[:, :])
            rw_t = cst.tile([128, K * 128], f32r)
            dma(out=rw_t, in_=rwt[:, :])
            gi_t = cst.tile([128, GROUPS], f32)
            dma(out=gi_t, in_=gind[:, :])
            git_t = cst.tile([GROUPS, 128], f32)
            dma(out=git_t, in_=gindT[:, :])
            rgb_t = cst.tile([128, 2], f32)
            dma(out=rgb_t, in_=rgb[:, :])
            m1_t = cst.tile([128, HID], f32)
            dma(out=m1_t, in_=m1wt[:, :])
            lnp_t = cst.tile([128, 3], f32)
            dma(out=lnp_t, in_=lnp[:, :])
            m2_t = cst.tile([128, CTX], f32)
            dma(out=m2_t, in_=m2wt[:, :])
            b2_t = cst.tile([CTX, 1], f32)
            dma(out=b2_t, in_=b2[:, :])
            dm_t = cst.tile([DIN, BPC], f32)
            dma(out=dm_t, in_=demoT[:, :])
            d1_t = cst.tile([DIN, 2 * DEMB], f32)
            dma(out=d1_t, in_=d1wt[:, :])
            dlnp_t = cst.tile([2 * DEMB, 3], f32)
            dma(out=dlnp_t, in_=dlnp[:, :])
            d2_t = cst.tile([2 * DEMB, DEMB], f32)
            dma(out=d2_t, in_=d2wt[:, :])
            db2_t = cst.tile([DEMB, 1], f32)
            dma(out=db2_t, in_=db2[:, :])
            gw_t = cst.tile([CTX + DEMB, E], f32)
            dma(out=gw_t, in_=gwt[:, :])
            gb_t = cst.tile([E, 1], f32)
            dma(out=gb_t, in_=gbi[:, :])
            ones_c = cst.tile([128, 1], f32)
            nc.vector.memset(ones_c, 1.0)
            ones_r = cst.tile([1, 128], f32)
            nc.vector.memset(ones_r, 1.0)
            eps_c = cst.tile([128, 1], f32)
            nc.vector.memset(eps_c, EPS)

            assert nc.vector.BN_STATS_FMAX >= 512

            # ---- demo path (independent; early so its Sqrt batches with
            # the GroupNorm Sqrt in one ACT table-set residency)
            psd1 = sps.tile([2 * DEMB, BPC], f32, tag="sp")
            nc.tensor.matmul(psd1, lhsT=d1_t, rhs=dm_t, start=True, stop=True)
            dln = wrk.tile([2 * DEMB, 2 * BPC], f32, tag="dln")
            nc.vector.tensor_scalar_add(
                out=dln[:, 0:BPC], in0=psd1, scalar1=dlnp_t[:, 0:1]
            )
            nc.scalar.activation(
                out=dln[:, BPC : 2 * BPC], in_=dln[:, 0:BPC], func=FT.Square
            )
            psds = sps.tile([1, 2 * BPC], f32, tag="sp")
            nc.tensor.matmul(
                psds, lhsT=ones_c[0 : 2 * DEMB, :], rhs=dln, start=True, stop=True
            )
            dst = wrk.tile([1, 2 * BPC], f32, tag="dst")
            nc.vector.tensor_scalar_mul(
                out=dst, in0=psds, scalar1=1.0 / (2 * DEMB)
            )
            dmsq = wrk.tile([1, BPC], f32, tag="dmsq")
            nc.vector.tensor_mul(dmsq, dst[:, 0:BPC], dst[:, 0:BPC])
            nc.vector.tensor_sub(dst[:, BPC : 2 * BPC], dst[:, BPC : 2 * BPC], dmsq)
            nc.scalar.activation(
                out=dst[:, BPC : 2 * BPC],
                in_=dst[:, BPC : 2 * BPC],
                func=FT.Sqrt,
                bias=eps_c[0:1, :],
            )
            nc.vector.reciprocal(out=dst[:, BPC : 2 * BPC], in_=dst[:, BPC : 2 * BPC])
            # ---- router conv round 1: stats
            stats = [stp.tile([128, NTT, 6], f32, tag=f"st{b}", name=f"stats{b}") for b in range(BPC)]
            for tt in range(NTT):
                pss = [cps.tile([128, 512], f32, tag="conv", name=f"cps{b}") for b in range(BPC)]
                for k in range(K):
                    for b in range(BPC):
                        nc.tensor.matmul(
                            pss[b],
                            lhsT=rw_t[32 * b : 32 * b + C, 128 * k : 128 * (k + 1)],
                            rhs=x_t[32 * b : 32 * b + C, tt * 512 + k : tt * 512 + k + 512],
                            start=(k == 0),
                            stop=(k == K - 1),
                            tile_position=(32 * b, 0),
                        )
                for b in range(BPC):
                    nc.vector.bn_stats(out=stats[b][:, tt, :], in_=pss[b])
            mvs = wrk.tile([128, BPC, 2], f32, tag="mvs")
            sums = wrk.tile([128, 2 * BPC], f32, tag="sums")
            tmpv = wrk.tile([128, BPC], f32, tag="tmpv")
            for b in range(BPC):
                nc.vector.bn_aggr(out=mvs[:, b, :], in_=stats[b])
            sums_v = sums.rearrange("p (b two) -> p b two", two=2)
            nc.vector.tensor_mul(tmpv, mvs[:, :, 0], mvs[:, :, 0])
            nc.vector.tensor_add(tmpv, mvs[:, :, 1], tmpv)
            nc.vector.tensor_scalar_mul(
                out=sums_v[:, :, 0], in0=mvs[:, :, 0], scalar1=float(T)
            )
            nc.vector.tensor_scalar_mul(
                out=sums_v[:, :, 1], in0=tmpv, scalar1=float(T)
            )
            psg = sps.tile([GROUPS, 2 * BPC], f32, tag="sp")
            nc.tensor.matmul(psg, lhsT=gi_t, rhs=sums, start=True, stop=True)
            # group mean / rstd  (bcin: cols 0:B mean, B:2B rstd)
            bcin = wrk.tile([GROUPS, 2 * BPC], f32, tag="bcin")
            psg_v = psg.rearrange("p (b two) -> p b two", two=2)
            nden = 1.0 / float(GSZ * T)
            nc.vector.tensor_scalar_mul(
                out=bcin[:, 0:BPC], in0=psg_v[:, :, 0], scalar1=nden
            )
            nc.vector.tensor_scalar_mul(
                out=bcin[:, BPC : 2 * BPC], in0=psg_v[:, :, 1], scalar1=nden
            )
            gmsq = wrk.tile([GROUPS, BPC], f32, tag="gmsq")
            nc.vector.tensor_mul(gmsq, bcin[:, 0:BPC], bcin[:, 0:BPC])
            nc.vector.tensor_sub(bcin[:, BPC : 2 * BPC], bcin[:, BPC : 2 * BPC], gmsq)
            nc.scalar.activation(
                out=bcin[:, BPC : 2 * BPC],
                in_=bcin[:, BPC : 2 * BPC],
                func=FT.Sqrt,
                bias=eps_c[0:GROUPS, :],
            )
            nc.vector.reciprocal(out=bcin[:, BPC : 2 * BPC], in_=bcin[:, BPC : 2 * BPC])
            psbc = sps.tile([128, 2 * BPC], f32, tag="sp")
            nc.tensor.matmul(psbc, lhsT=git_t, rhs=bcin, start=True, stop=True)
            scl = wrk.tile([128, BPC], f32, tag="scl")
            nc.vector.tensor_scalar_mul(
                out=scl, in0=psbc[:, BPC : 2 * BPC], scalar1=rgb_t[:, 0:1]
            )
            bia = wrk.tile([128, BPC], f32, tag="bia")
            nc.vector.tensor_mul(bia, psbc[:, 0:BPC], scl)
            nc.vector.tensor_scalar(
                out=bia,
                in0=bia,
                scalar1=-1.0,
                scalar2=rgb_t[:, 1:2],
                op0=AL.mult,
                op1=AL.add,
            )

            # ---- demo tail (gelu batches with conv gelus in one table set)
            psdb = sps.tile([2 * DEMB, 2 * BPC], f32, tag="sp")
            nc.tensor.matmul(
                psdb, lhsT=ones_r[:, 0 : 2 * DEMB], rhs=dst, start=True, stop=True
            )
            dy = wrk.tile([2 * DEMB, BPC], f32, tag="dy")
            nc.vector.tensor_sub(dy, dln[:, 0:BPC], psdb[:, 0:BPC])
            nc.vector.tensor_mul(dy, dy, psdb[:, BPC : 2 * BPC])
            nc.vector.tensor_scalar(
                out=dy,
                in0=dy,
                scalar1=dlnp_t[:, 1:2],
                scalar2=dlnp_t[:, 2:3],
                op0=AL.mult,
                op1=AL.add,
            )
            nc.scalar.activation(out=dy, in_=dy, func=FT.Gelu)
            psd2 = sps.tile([DEMB, BPC], f32, tag="sp")
            nc.tensor.matmul(psd2, lhsT=d2_t, rhs=dy, start=True, stop=True)
            catT = wrk.tile([CTX + DEMB, BPC], f32, tag="cat")
            nc.vector.tensor_scalar_add(
                out=catT[CTX : CTX + DEMB, :], in0=psd2, scalar1=db2_t
            )


            # ---- router conv round 2: gelu + GAP (accum)
            gacc = wrk.tile([128, BPC, NTT], f32, tag="gacc")
            for tt in range(NTT):
                pss = [cps.tile([128, 512], f32, tag="conv", name=f"cp2{b}") for b in range(BPC)]
                for k in range(K):
                    for b in range(BPC):
                        nc.tensor.matmul(
                            pss[b],
                            lhsT=rw_t[32 * b : 32 * b + C, 128 * k : 128 * (k + 1)],
                            rhs=x_t[32 * b : 32 * b + C, tt * 512 + k : tt * 512 + k + 512],
                            start=(k == 0),
                            stop=(k == K - 1),
                            tile_position=(32 * b, 0),
                        )
                for b in range(BPC):
                    hsc = scr.tile([128, 512], f32, tag="hsc")
                    nc.scalar.activation(
                        out=hsc,
                        in_=pss[b],
                        func=FT.Gelu,
                        scale=scl[:, b : b + 1],
                        bias=bia[:, b : b + 1],
                        accum_out=gacc[:, b, tt : tt + 1],
                    )
            rT = wrk.tile([128, BPC], f32, tag="rT")
            nc.vector.tensor_reduce(out=rT, in_=gacc, axis=AX.X, op=AL.add)
            nc.vector.tensor_scalar_mul(out=rT, in0=rT, scalar1=1.0 / float(T))

            # ---- MLP: y1 = gelu(LN(r @ m1 + b1)); out2 = y1 @ m2 + b2
            psm1 = sps.tile([HID, BPC], f32, tag="sp")
            nc.tensor.matmul(psm1, lhsT=m1_t, rhs=rT, start=True, stop=True)
            lin = wrk.tile([HID, 2 * BPC], f32, tag="lin")
            nc.vector.tensor_scalar_add(
                out=lin[:, 0:BPC], in0=psm1, scalar1=lnp_t[:, 0:1]
            )
            nc.scalar.activation(
                out=lin[:, BPC : 2 * BPC], in_=lin[:, 0:BPC], func=FT.Square
            )
            psls = sps.tile([1, 2 * BPC], f32, tag="sp")
            nc.tensor.matmul(psls, lhsT=ones_c, rhs=lin, start=True, stop=True)
            lst = wrk.tile([1, 2 * BPC], f32, tag="lst")
            nc.vector.tensor_scalar_mul(out=lst, in0=psls, scalar1=1.0 / float(HID))
            lmsq = wrk.tile([1, BPC], f32, tag="lmsq")
            nc.vector.tensor_mul(lmsq, lst[:, 0:BPC], lst[:, 0:BPC])
            nc.vector.tensor_sub(lst[:, BPC : 2 * BPC], lst[:, BPC : 2 * BPC], lmsq)
            nc.scalar.activation(
                out=lst[:, BPC : 2 * BPC],
                in_=lst[:, BPC : 2 * BPC],
                func=FT.Sqrt,
                bias=eps_c[0:1, :],
            )
            nc.vector.reciprocal(out=lst[:, BPC : 2 * BPC], in_=lst[:, BPC : 2 * BPC])
            pslb = sps.tile([HID, 2 * BPC], f32, tag="sp")
            nc.tensor.matmul(pslb, lhsT=ones_r, rhs=lst, start=True, stop=True)
            y1 = wrk.tile([HID, BPC], f32, tag="y1")
            nc.vector.tensor_sub(y1, lin[:, 0:BPC], pslb[:, 0:BPC])
            nc.vector.tensor_mul(y1, y1, pslb[:, BPC : 2 * BPC])
            nc.vector.tensor_scalar(
                out=y1,
                in0=y1,
                scalar1=lnp_t[:, 1:2],
                scalar2=lnp_t[:, 2:3],
                op0=AL.mult,
                op1=AL.add,
            )
            nc.scalar.activation(out=y1, in_=y1, func=FT.Gelu)
            psm2 = sps.tile([CTX, BPC], f32, tag="sp")
            nc.tensor.matmul(psm2, lhsT=m2_t, rhs=y1, start=True, stop=True)
            nc.vector.tensor_scalar_add(out=catT[0:CTX, :], in0=psm2, scalar1=b2_t)

            # ---- gate logits
            psgt = sps.tile([E, BPC], f32, tag="sp")
            nc.tensor.matmul(psgt, lhsT=gw_t, rhs=catT, start=True, stop=True)
            lg = wrk.tile([E, BPC], f32, tag="lg")
            nc.vector.tensor_scalar_add(out=lg, in0=psgt, scalar1=gb_t)
            dma(out=logout[:, :], in_=lg)

    return _split_multiwait(nc)


def _build_b():
    import concourse.bass as bass
    import concourse.tile as tile
    from concourse import mybir

    f32 = mybir.dt.float32
    f32r = mybir.dt.float32r
    FT = mybir.ActivationFunctionType
    AL = mybir.AluOpType

    NS = 3  # slots: shared, expert0, expert1
    NC12 = BPC * NS

    nc = bass.Bass()
    xin = nc.dram_tensor("xin", [128, TPAD], f32r, kind="ExternalInput")
    wpk = nc.dram_tensor("wpk", [128, NS * K * 128], f32r, kind="ExternalInput")
    gind = nc.dram_tensor("gind", [128, GROUPS], f32, kind="ExternalInput")
    gindT = nc.dram_tensor("gindT", [GROUPS, 128], f32, kind="ExternalInput")
    gnw = nc.dram_tensor("gnw", [128, NC12], f32, kind="ExternalInput")
    gnb = nc.dram_tensor("gnb", [128, NC12], f32, kind="ExternalInput")
    wv = nc.dram_tensor("wv", [128, NC12], f32, kind="ExternalInput")
    outd = nc.dram_tensor("out", [BPC, 128, T], f32, kind="ExternalOutput")

    with tile.TileContext(nc) as tc:
        with (
            tc.tile_pool(name="const", bufs=1) as cst,
            tc.tile_pool(name="stats", bufs=1) as stp,
            tc.tile_pool(name="work", bufs=1) as wrk,
            tc.tile_pool(name="ysc", bufs=14) as ysc,
            tc.tile_pool(name="osb", bufs=3) as osp,
            tc.tile_pool(name="cps", bufs=4, space="PSUM") as cps,
        ):
            dma = nc.gpsimd.dma_start
            x_t = cst.tile([128, TPAD], f32r)
            dma(out=x_t, in_=xin[:, :])
            w_t = cst.tile([128, NS * K * 128], f32r)
            dma(out=w_t, in_=wpk[:, :])
            gi_t = cst.tile([128, GROUPS], f32)
            dma(out=gi_t, in_=gind[:, :])
            git_t = cst.tile([GROUPS, 128], f32)
            dma(out=git_t, in_=gindT[:, :])
            gnw_t = cst.tile([128, NC12], f32)
            dma(out=gnw_t, in_=gnw[:, :])
            gnb_t = cst.tile([128, NC12], f32)
            dma(out=gnb_t, in_=gnb[:, :])
            wv_t = cst.tile([128, NC12], f32)
            dma(out=wv_t, in_=wv[:, :])
            eps_c = cst.tile([GROUPS, 1], f32)
            nc.vector.memset(eps_c, EPS)

            def conv(ps, s, b, ch):
                # accumulate 5 shifted matmuls for slot s, batch b over a
                # (128,1024) chunk ch; k==2 adds the ones-row (conv bias)
                for tth in range(2):
                    tt0 = ch * 1024 + tth * 512
                    for k in range(K):
                        rows = C + 1 if k == 2 else C
                        nc.tensor.matmul(
                            ps[:, tth * 512 : tth * 512 + 512],
                            lhsT=w_t[
                                32 * b : 32 * b + rows,
                                (s * K + k) * 128 : (s * K + k + 1) * 128,
                            ],
                            rhs=x_t[32 * b : 32 * b + rows, tt0 + k : tt0 + k + 512],
                            start=(k == 0),
                            stop=(k == K - 1),
                            tile_position=(32 * b, 0),
                        )

            # ---- round 1: stats
            stats = [stp.tile([128, NTT, 6], f32, tag=f"st{c}", name=f"stats{c}") for c in range(NC12)]
            for ch in range(NCH):
                for s in range(NS):
                    pss = [cps.tile([128, 1024], f32, tag="conv", name=f"cps{b}") for b in range(BPC)]
                    for tth in range(2):
                        tt0 = ch * 1024 + tth * 512
                        for k in range(K):
                            rows = C + 1 if k == 2 else C
                            for b in range(BPC):
                                nc.tensor.matmul(
                                    pss[b][:, tth * 512 : tth * 512 + 512],
                                    lhsT=w_t[
                                        32 * b : 32 * b + rows,
                                        (s * K + k) * 128 : (s * K + k + 1) * 128,
                                    ],
                                    rhs=x_t[32 * b : 32 * b + rows, tt0 + k : tt0 + k + 512],
                                    start=(k == 0),
                                    stop=(k == K - 1),
                                    tile_position=(32 * b, 0),
                                )
                    for b in range(BPC):
                        c = b * NS + s
                        nc.vector.bn_stats(
                            out=stats[c][:, 2 * ch, :], in_=pss[b][:, 0:512]
                        )
                        nc.vector.bn_stats(
                            out=stats[c][:, 2 * ch + 1, :], in_=pss[b][:, 512:1024]
                        )
            mvs = wrk.tile([128, NC12, 2], f32, tag="mvs")
            sums = wrk.tile([128, 2 * NC12], f32, tag="sums")
            tmpv = wrk.tile([128, NC12], f32, tag="tmpv")
            for c in range(NC12):
                nc.vector.bn_aggr(out=mvs[:, c, :], in_=stats[c])
            sums_v = sums.rearrange("p (c two) -> p c two", two=2)
            nc.vector.tensor_mul(tmpv, mvs[:, :, 0], mvs[:, :, 0])
            nc.vector.tensor_add(tmpv, mvs[:, :, 1], tmpv)
            nc.vector.tensor_scalar_mul(
                out=sums_v[:, :, 0], in0=mvs[:, :, 0], scalar1=float(T)
            )
            nc.vector.tensor_scalar_mul(
                out=sums_v[:, :, 1], in0=tmpv, scalar1=float(T)
            )
            psg = cps.tile([GROUPS, 2 * NC12], f32, tag="conv")
            nc.tensor.matmul(psg, lhsT=gi_t, rhs=sums, start=True, stop=True)
            bcin = wrk.tile([GROUPS, 2 * NC12], f32, tag="bcin")
            psg_v = psg.rearrange("p (c two) -> p c two", two=2)
            nden = 1.0 / float(GSZ * T)
            nc.vector.tensor_scalar_mul(
                out=bcin[:, 0:NC12], in0=psg_v[:, :, 0], scalar1=nden
            )
            nc.vector.tensor_scalar_mul(
                out=bcin[:, NC12 : 2 * NC12], in0=psg_v[:, :, 1], scalar1=nden
            )
            gmsq = wrk.tile([GROUPS, NC12], f32, tag="gmsq")
            nc.vector.tensor_mul(gmsq, bcin[:, 0:NC12], bcin[:, 0:NC12])
            nc.vector.tensor_sub(
                bcin[:, NC12 : 2 * NC12], bcin[:, NC12 : 2 * NC12], gmsq
            )
            nc.scalar.activation(
                out=bcin[:, NC12 : 2 * NC12],
                in_=bcin[:, NC12 : 2 * NC12],
                func=FT.Sqrt,
                bias=eps_c,
            )
            nc.vector.reciprocal(
                out=bcin[:, NC12 : 2 * NC12], in_=bcin[:, NC12 : 2 * NC12]
            )
            psbc = cps.tile([128, 2 * NC12], f32, tag="conv")
            nc.tensor.matmul(psbc, lhsT=git_t, rhs=bcin, start=True, stop=True)
            scl = wrk.tile([128, NC12], f32, tag="scl")
            nc.vector.tensor_mul(scl, psbc[:, NC12 : 2 * NC12], gnw_t)
            bia = wrk.tile([128, NC12], f32, tag="bia")
            nc.vector.tensor_mul(bia, psbc[:, 0:NC12], scl)
            nc.vector.tensor_scalar_mul(out=bia, in0=bia, scalar1=-1.0)
            nc.vector.tensor_add(bia, gnb_t, bia)

            # ---- round 2: recompute conv, gelu, weighted combine, store
            yss = {}
            for ch in range(NCH):
                for s in range(NS):
                    pss = [cps.tile([128, 1024], f32, tag="conv", name=f"cp2{b}") for b in range(BPC)]
                    for tth in range(2):
                        tt0 = ch * 1024 + tth * 512
                        for k in range(K):
                            rows = C + 1 if k == 2 else C
                            for b in range(BPC):
                                nc.tensor.matmul(
                                    pss[b][:, tth * 512 : tth * 512 + 512],
                                    lhsT=w_t[
                                        32 * b : 32 * b + rows,
                                        (s * K + k) * 128 : (s * K + k + 1) * 128,
                                    ],
                                    rhs=x_t[32 * b : 32 * b + rows, tt0 + k : tt0 + k + 512],
                                    start=(k == 0),
                                    stop=(k == K - 1),
                                    tile_position=(32 * b, 0),
                                )
                    for b in range(BPC):
                        c = b * NS + s
                        yt = ysc.tile([128, 1024], f32, tag="y", name=f"y{s}_{b}")
                        nc.scalar.activation(
                            out=yt,
                            in_=pss[b],
                            func=FT.Gelu,
                            scale=scl[:, c : c + 1],
                            bias=bia[:, c : c + 1],
                        )
                        yss[(s, b)] = yt
                for b in range(BPC):
                    c = b * NS
                    # spread the 3-way weighted sum across ACT / Pool / DVE
                    t1 = osp.tile([128, 1024], f32, tag="t1", name=f"t1_{b}")
                    nc.scalar.activation(
                        out=t1,
                        in_=yss[(1, b)],
                        func=FT.Identity,
                        scale=wv_t[:, c + 1 : c + 2],
                    )
                    t2 = osp.tile([128, 1024], f32, tag="t2", name=f"t2_{b}")
                    nc.gpsimd.tensor_add(t2, yss[(0, b)], t1)
                    ob = osp.tile([128, 1024], f32, tag="ob", name=f"ob{b}")
                    nc.vector.scalar_tensor_tensor(
                        out=ob,
                        in0=yss[(2, b)],
                        scalar=wv_t[:, c + 2 : c + 3],
                        in1=t2,
                        op0=AL.mult,
                        op1=AL.add,
                    )
                    dma(
                        out=outd[b, :, ch * 1024 : ch * 1024 + 1024],
                        in_=ob,
                    )

    return _split_multiwait(nc)


def _prep_a_inmaps(inputs):
    f = np.float32
    x = np.asarray(inputs["x"], f)
    demo = np.asarray(inputs["demo"], f)
    rw = np.asarray(inputs["rw"], f)

    gind = np.zeros((128, GROUPS), f)
    for cch in range(128):
        gind[cch, cch // GSZ] = 1.0
    gindT = np.ascontiguousarray(gind.T)

    rwt = np.zeros((128, K * 128), f)
    blk = np.ascontiguousarray(rw.transpose(1, 2, 0).reshape(C, K * 128))
    for b in range(BPC):
        rwt[32 * b : 32 * b + C, :] = blk

    rgb = np.stack([np.asarray(inputs["rg"], f), np.asarray(inputs["rb"], f)], 1)
    m1wt = np.ascontiguousarray(np.asarray(inputs["m1_w"], f).T)
    lnp = np.stack(
        [
            np.asarray(inputs["m1_b"], f),
            np.asarray(inputs["ln_g"], f),
            np.asarray(inputs["ln_b"], f),
        ],
        1,
    )
    m2wt = np.ascontiguousarray(np.asarray(inputs["m2_w"], f).T)
    b2 = np.asarray(inputs["m2_b"], f)[:, None]
    d1wt = np.ascontiguousarray(np.asarray(inputs["d1_w"], f).T)
    dlnp = np.stack(
        [
            np.asarray(inputs["d1_b"], f),
            np.asarray(inputs["dln_g"], f),
            np.asarray(inputs["dln_b"], f),
        ],
        1,
    )
    d2wt = np.ascontiguousarray(np.asarray(inputs["d2_w"], f).T)
    db2 = np.asarray(inputs["d2_b"], f)[:, None]
    gwt = np.ascontiguousarray(np.asarray(inputs["g_w"], f).T)
    gbi = np.asarray(inputs["g_b"], f)[:, None]

    xas = []
    in_maps = []
    for cid in range(NCORES):
        xa = np.zeros((128, TPAD), f)
        for b in range(BPC):
            gb = cid * BPC + b
            xa[32 * b : 32 * b + C, 2 : 2 + T] = x[gb]
            xa[32 * b + C, :] = 1.0
        xas.append(xa)
        demoT = np.ascontiguousarray(demo[cid * BPC : (cid + 1) * BPC].T)
        in_maps.append(
            dict(
                xin=xa,
                rwt=rwt,
                gind=gind,
                gindT=gindT,
                rgb=rgb,
                m1wt=m1wt,
                lnp=lnp,
                m2wt=m2wt,
                b2=b2,
                demoT=demoT,
                d1wt=d1wt,
                dlnp=dlnp,
                d2wt=d2wt,
                db2=db2,
                gwt=gwt,
                gbi=gbi,
            )
        )
    return in_maps, xas, gind, gindT


def _prep_b_inmaps(inputs, logits, xas, gind, gindT):
    f = np.float32
    sw = np.asarray(inputs["sw"], f)
    sb = np.asarray(inputs["sb"], f)
    sg = np.asarray(inputs["sg"], f)
    sbt = np.asarray(inputs["sbt"], f)
    ew = np.asarray(inputs["ew"], f)
    eb = np.asarray(inputs["eb"], f)
    eg = np.asarray(inputs["eg"], f)
    ebt = np.asarray(inputs["ebt"], f)

    # softmax + top-2 + renormalize (mirrors the reference gate math)
    lm = logits - logits.max(-1, keepdims=True)
    e_ = np.exp(lm, dtype=f)
    ws = e_ / e_.sum(-1, keepdims=True)
    order = np.argsort(-ws, axis=-1, kind="stable")[:, :2]
    w01 = np.take_along_axis(ws, order, axis=-1)
    hard = w01 / (w01.sum(-1, keepdims=True) + f(1e-9))

    NS = 3
    NC12 = BPC * NS
    in_maps = []
    for cid in range(NCORES):
        wpkc = np.zeros((128, NS * K * 128), f)
        gnwc = np.zeros((128, NC12), f)
        gnbc = np.zeros((128, NC12), f)
        wvc = np.zeros((128, NC12), f)
        for b in range(BPC):
            gb = cid * BPC + b
            for s in range(NS):
                if s == 0:
                    W, cb, gg, bb, wval = sw, sb, sg, sbt, 1.0
                else:
                    ei = int(order[gb, s - 1])
                    W, cb, gg, bb = ew[ei], eb[ei], eg[ei], ebt[ei]
                    wval = float(hard[gb, s - 1])
                blk = np.ascontiguousarray(W.transpose(1, 2, 0).reshape(C, K * 128))
                wpkc[32 * b : 32 * b + C, s * K * 128 : (s + 1) * K * 128] = blk
                # conv bias rides the ones-row, folded into the k==2 matmul
                wpkc[32 * b + C, (s * K + 2) * 128 : (s * K + 3) * 128] = cb
                cix = b * NS + s
                gnwc[:, cix] = gg
                gnbc[:, cix] = bb
                wvc[:, cix] = wval
        in_maps.append(
            dict(
                xin=xas[cid],
                wpk=wpkc,
                gind=gind,
                gindT=gindT,
                gnw=gnwc,
                gnb=gnbc,
                wv=wvc,
            )
        )
    return in_maps


def _run(nc, in_maps, trace=False):
    from concourse.bass_utils import run_bass_kernel_spmd

    return run_bass_kernel_spmd(nc, in_maps, list(range(NCORES)), trace=trace)


def kernel(**inputs):
    import os

    trace = bool(int(os.environ.get("MOE_TRACE", "0")))
    if "a" not in _built:
        _built["a"] = _build_a()
        _built["b"] = _build_b()

    in_a, xas, gind, gindT = _prep_a_inmaps(inputs)
    res_a = _run(_built["a"], in_a, trace=trace)
    logits = np.zeros((B, E), np.float32)
    for cid in range(NCORES):
        lt = res_a.results[cid]["logitsT"]  # (E, BPC)
        logits[cid * BPC : (cid + 1) * BPC, :] = lt.T

    in_b = _prep_b_inmaps(inputs, logits, xas, gind, gindT)
    res_b = _run(_built["b"], in_b, trace=trace)
    out = np.concatenate([res_b.results[cid]["out"] for cid in range(NCORES)], 0)

    kernel.last_exec_ns = (res_a.exec_time_ns or 0) + (res_b.exec_time_ns or 0)
    kernel.last_results = (res_a, res_b)
    kernel.last_logits = logits
    return out



# revision 27
# speedup vs baseline: 2.5341x; 2.5341x over previous
"""Trainium2 Bass kernel for nn_ContrastiveEncoderMOE.

Strategy: data-parallel over batch (4 batches per core, 8 cores, no
collectives), two launches:

  A) router: im2col conv (f32r, contraction 81 = 5 taps x 16 ch + bias row,
     so each 512-col output tile is ONE matmul) -> GroupNorm with stats
     measured on-device (sum via Pool accum pass + PE matvec, sum-of-squares
     via one-pass DVE tensor_tensor_reduce on the PSUM-resident conv out)
     -> GELU+GAP (ACT accum_out) -> MLP(+LN) -> concat demo embedding ->
     gate logits (per-core (8,4)).
  host: softmax + top-2 + renormalize on (32,8); pack the 2 selected
     experts' conv weights per batch (control plane only).
  B) shared conv + 2 selected expert convs per batch in bf16. GroupNorm
     stats come from the Gram matrix G = xcol @ xcol^T computed on the PE
     from a host-pretransposed copy of the im2col matrix: sum h = w^T G e80
     (ones row), sum h^2 = w^T G w (via G@W on PE, W.M elementwise on DVE,
     ones-matvec on PE). Conv output stays in PSUM; ACT applies the folded
     GroupNorm affine + GELU in one pass; 3-slot weighted combine split
     across Pool (stt1) and DVE (stt2). All DMA issued from the SP engine.
"""

import numpy as np
import ml_dtypes

BF = ml_dtypes.bfloat16

B, C, T = 32, 16, 2048
E, CO, K = 8, 128, 5
HID, CTX, DIN, DEMB = 128, 64, 8, 16
GROUPS = 8
NCORES = 8
BPC = B // NCORES          # batches per core
KC = K * C                 # 80
KC1 = KC + 1               # 81 (bias/ones row)
NS = 3                     # slots: shared, expert0, expert1
NC12 = NS * BPC            # 12
EPS = 1e-5
GSZ = CO // GROUPS         # 16
NCHK = T // 128            # 16 transpose chunks
XTW = NCHK * KC1           # 1296 columns per batch in xcolT

_built = {}


def _split_multiwait(nc, max_waits=1):
    # The pinned walrus rejects >1 sync-wait on one instruction
    # ("Too many sync wait commands"); hoist excess waits onto
    # same-engine NOPs inserted just before.
    from concourse import mybir

    for f in nc.m.functions:
        for blk in f.blocks:
            out = []
            for inst in blk.instructions:
                si = getattr(inst, "sync_info", None)
                if si is not None and si.on_wait and len(si.on_wait) > max_waits:
                    waits = list(si.on_wait)
                    cnt = 0
                    while len(waits) > max_waits:
                        chunk, waits = waits[:max_waits], waits[max_waits:]
                        nop = mybir.InstNoOp(
                            name=f"{inst.name}-mw{cnt}",
                            engine=inst.engine,
                            bass_nofuse=True,
                            sync_info=mybir.SyncInfo(on_wait=chunk, on_update=[]),
                        )
                        out.append(nop)
                        cnt += 1
                    inst.sync_info = mybir.SyncInfo(
                        on_wait=waits, on_update=list(si.on_update)
                    )
                out.append(inst)
            blk.instructions[:] = out
    return nc


# --------------------------------------------------------------------------
# kernel A: router -> gate logits
# --------------------------------------------------------------------------

# packed const layout (columns of the [128, 399] f32 "mlpp" tensor)
_M1 = 0          # [128, 0:128]   m1_w.T
_LNP = 128       # [128, 128:131] m1_b | ln_g | ln_b
_M2 = 131        # [128, 131:195] m2_w.T
_B2 = 195        # [64, 195:196]
_DM = 196        # [8, 196:200]   demo.T
_D1 = 200        # [8, 200:232]   d1_w.T
_DLN = 232       # [32, 232:235]  d1_b | dln_g | dln_b
_D2 = 235        # [32, 235:251]  d2_w.T
_DB2 = 251       # [16, 251:252]
_GW = 252        # [80, 252:260]  g_w.T
_GB = 260        # [8, 260:261]
_GI8 = 261       # [128, 261:269] one-hot group matrix
_RGB = 269       # [128, 269:271] rg | rb
_GIT = 271       # [8, 271:399]   one-hot group matrix transposed
_MLPW = 399


def _build_a():
    import concourse.bass as bass
    import concourse.tile as tile
    from concourse import mybir

    f32 = mybir.dt.float32
    f32r = mybir.dt.float32r
    FT = mybir.ActivationFunctionType
    AL = mybir.AluOpType

    nc = bass.Bass()
    xcf = nc.dram_tensor("xcf", [KC1, BPC * T], f32r, kind="ExternalInput")
    rwc = nc.dram_tensor("rwc", [KC1, HID], f32r, kind="ExternalInput")
    mlpp = nc.dram_tensor("mlpp", [128, _MLPW], f32, kind="ExternalInput")
    logout = nc.dram_tensor("logitsT", [E, BPC], f32, kind="ExternalOutput")

    with tile.TileContext(nc) as tc:
        with (
            tc.tile_pool(name="const", bufs=1) as cst,
            tc.tile_pool(name="work", bufs=1) as wrk,
            tc.tile_pool(name="scr", bufs=4) as scr,
            tc.tile_pool(name="hps", bufs=3, space="PSUM") as hps,
            tc.tile_pool(name="aps", bufs=1, space="PSUM") as aps,
        ):
            dma = nc.sync.dma_start
            xc_t = cst.tile([KC1, BPC * T], f32r)
            rw_t = cst.tile([KC1, HID], f32r)
            mp_t = cst.tile([128, _MLPW], f32)
            dma(out=xc_t[:, 0:512], in_=xcf[:, 0:512])
            dma(out=mp_t[:, _DM:_GB], in_=mlpp[:, _DM:_GB])  # demo params early
            dma(out=rw_t, in_=rwc[:, :])
            dma(out=xc_t[:, 512:1024], in_=xcf[:, 512:1024])
            dma(out=xc_t[:, 1024:T], in_=xcf[:, 1024:T])
            dma(out=mp_t[:, 0:_DM], in_=mlpp[:, 0:_DM])
            dma(out=mp_t[:, _GB:_MLPW], in_=mlpp[:, _GB:_MLPW])
            for b in range(1, BPC):
                dma(out=xc_t[:, b * T : (b + 1) * T], in_=xcf[:, b * T : (b + 1) * T])

            ones_c = cst.tile([128, 1], f32)
            nc.vector.memset(ones_c, 1.0)
            ones_r = cst.tile([1, 128], f32)
            nc.vector.memset(ones_r, 1.0)
            eps_c = cst.tile([128, 1], f32)
            nc.vector.memset(eps_c, EPS)

            # PSUM arena (one bank), hand-sliced
            arena = aps.tile([128, 512], f32, tag="arena")
            s_ps = arena[:, 0:4]         # batched S1 matvec out [128, BPC]
            gs_ps = arena[0:8, 8:20]     # per-batch [8,3] at 8+3b
            bc_ps = arena[:, 20:28]      # per-batch [128,2] at 20+2b
            psm1 = arena[:, 28:32]
            psls = arena[0:1, 32:40]
            pslb = arena[:, 40:48]
            psm2 = arena[0:CTX, 48:52]
            psgt = arena[0:E, 52:56]
            psd1 = arena[0 : 2 * DEMB, 56:60]
            psds = arena[0:1, 60:68]
            psdb = arena[0 : 2 * DEMB, 68:76]
            psd2 = arena[0:DEMB, 76:80]

            # ---- demo path (emitted mid-stream once mlpp has landed)
            dm_t = mp_t[0:DIN, _DM : _DM + BPC]
            d1_t = mp_t[0:DIN, _D1 : _D1 + 2 * DEMB]
            dlnp = mp_t[0 : 2 * DEMB, _DLN : _DLN + 3]
            d2_t = mp_t[0 : 2 * DEMB, _D2 : _D2 + DEMB]
            db2 = mp_t[0:DEMB, _DB2 : _DB2 + 1]
            catT = wrk.tile([CTX + DEMB, BPC], f32, tag="cat")

            dln = wrk.tile([2 * DEMB, 2 * BPC], f32, tag="dln")
            dst = wrk.tile([1, 2 * BPC], f32, tag="dst")

            def demo_0():
              nc.tensor.matmul(psd1, lhsT=d1_t, rhs=dm_t, start=True, stop=True)

            def demo_1():
              nc.vector.tensor_scalar_add(
                  out=dln[:, 0:BPC], in0=psd1, scalar1=dlnp[:, 0:1]
              )
              nc.scalar.activation(
                  out=dln[:, BPC : 2 * BPC],
                  in_=psd1,
                  func=FT.Square,
                  bias=dlnp[:, 0:1],
              )
              nc.tensor.matmul(
                  psds, lhsT=ones_c[0 : 2 * DEMB, :], rhs=dln, start=True, stop=True
              )

            def demo_2():
              nc.vector.tensor_scalar_mul(out=dst, in0=psds, scalar1=1.0 / (2 * DEMB))
              dmsq = wrk.tile([1, BPC], f32, tag="dmsq")
              nc.vector.tensor_mul(dmsq, dst[:, 0:BPC], dst[:, 0:BPC])
              nc.vector.tensor_sub(dst[:, BPC : 2 * BPC], dst[:, BPC : 2 * BPC], dmsq)
              nc.scalar.activation(
                  out=dst[:, BPC : 2 * BPC],
                  in_=dst[:, BPC : 2 * BPC],
                  func=FT.Sqrt,
                  bias=eps_c[0:1, :],
              )
              nc.vector.reciprocal(
                  out=dst[:, BPC : 2 * BPC], in_=dst[:, BPC : 2 * BPC]
              )
              nc.tensor.matmul(
                  psdb, lhsT=ones_r[:, 0 : 2 * DEMB], rhs=dst, start=True, stop=True
              )

            def demo_3():
              dy = wrk.tile([2 * DEMB, BPC], f32, tag="dy")
              nc.vector.tensor_sub(dy, dln[:, 0:BPC], psdb[:, 0:BPC])
              nc.vector.tensor_mul(dy, dy, psdb[:, BPC : 2 * BPC])
              nc.vector.tensor_scalar(
                  out=dy,
                  in0=dy,
                  scalar1=dlnp[:, 1:2],
                  scalar2=dlnp[:, 2:3],
                  op0=AL.mult,
                  op1=AL.add,
              )
              nc.scalar.activation(out=dy, in_=dy, func=FT.Gelu)
              nc.tensor.matmul(psd2, lhsT=d2_t, rhs=dy, start=True, stop=True)
              nc.vector.tensor_scalar_add(
                  out=catT[CTX : CTX + DEMB, :], in0=psd2, scalar1=db2
              )

            # ---- router conv + GN + GELU + GAP, per batch
            gi8 = mp_t[:, _GI8 : _GI8 + GROUPS]
            giT8 = mp_t[0:GROUPS, _GIT : _GIT + 128]
            rgb = mp_t[:, _RGB : _RGB + 2]
            agg = wrk.tile([GROUPS, 8 * BPC], f32, tag="agg")
            bcs = wrk.tile([128, 2 * BPC], f32, tag="bcs")
            sclA = wrk.tile([128, BPC], f32, tag="sclA")
            biaA = wrk.tile([128, BPC], f32, tag="biaA")
            tmpA = wrk.tile([128, BPC], f32, tag="tmpA")
            racc = wrk.tile([128, 2 * BPC], f32, tag="racc")
            bnst = wrk.tile([128, BPC, 4, 6], f32, tag="bnst")
            mvs = wrk.tile([128, BPC, 2], f32, tag="mvs")
            e2c = wrk.tile([128, BPC], f32, tag="e2c")
            tmv = wrk.tile([128, BPC], f32, tag="tmv")
            rw_r = rw_t[:, :]

            def round1(b):
                # conv halves -> bn_stats; tiles freed after stats
                for hh in range(2):
                    h = hps.tile([128, 1024], f32, tag="h", name=f"h{b}_{hh}")
                    for q in range(2):
                        c0 = b * T + hh * 1024 + q * 512
                        nc.tensor.matmul(
                            h[:, q * 512 : (q + 1) * 512],
                            lhsT=rw_r,
                            rhs=xc_t[:, c0 : c0 + 512],
                            start=True,
                            stop=True,
                        )
                    for q in range(2):
                        nc.vector.bn_stats(
                            out=bnst[:, b, 2 * hh + q, :],
                            in_=h[:, q * 512 : (q + 1) * 512],
                        )

            def aggr_front(b):
                # per-channel mean/var -> E[h], E[h^2] columns -> group reduce
                nc.vector.bn_aggr(out=mvs[:, b, :], in_=bnst[:, b])
                nc.vector.tensor_mul(tmv[:, b : b + 1], mvs[:, b, 0:1], mvs[:, b, 0:1])
                nc.vector.tensor_add(e2c[:, b : b + 1], mvs[:, b, 1:2], tmv[:, b : b + 1])
                gsb = gs_ps[:, 3 * b : 3 * b + 2]
                nc.tensor.matmul(
                    gsb[:, 0:1], lhsT=gi8, rhs=mvs[:, b, 0:1], start=True, stop=True
                )
                nc.tensor.matmul(
                    gsb[:, 1:2], lhsT=gi8, rhs=e2c[:, b : b + 1], start=True, stop=True
                )
                a0 = 8 * b
                gss = agg[:, a0 : a0 + 2]  # mu | e2 (group means)
                nc.vector.tensor_scalar_mul(out=gss, in0=gsb, scalar1=1.0 / float(GSZ))
                msq = agg[:, a0 + 4 : a0 + 5]
                nc.vector.tensor_mul(msq, gss[:, 0:1], gss[:, 0:1])
                var = agg[:, a0 + 5 : a0 + 6]
                nc.vector.tensor_sub(var, gss[:, 1:2], msq)
                nc.scalar.activation(
                    out=var, in_=var, func=FT.Sqrt, bias=eps_c[0:GROUPS, :]
                )
                nc.vector.reciprocal(out=var, in_=var)

            def aggr_back(b):
                a0 = 8 * b
                gss = agg[:, a0 : a0 + 2]
                var = agg[:, a0 + 5 : a0 + 6]
                bcb = bc_ps[:, 2 * b : 2 * b + 2]
                nc.tensor.matmul(
                    bcb[:, 0:1], lhsT=giT8, rhs=gss[:, 0:1], start=True, stop=True
                )
                nc.tensor.matmul(
                    bcb[:, 1:2], lhsT=giT8, rhs=var, start=True, stop=True
                )
                nc.vector.tensor_scalar_mul(
                    out=bcs[:, 2 * b : 2 * b + 2], in0=bcb, scalar1=1.0
                )
                nc.vector.tensor_mul(
                    sclA[:, b : b + 1], bcs[:, 2 * b + 1 : 2 * b + 2], rgb[:, 0:1]
                )
                nc.vector.tensor_mul(
                    tmpA[:, b : b + 1], bcs[:, 2 * b : 2 * b + 1], sclA[:, b : b + 1]
                )
                nc.vector.tensor_sub(biaA[:, b : b + 1], rgb[:, 1:2], tmpA[:, b : b + 1])

            def round2(b):
                # recompute conv, then GELU + GAP accumulate
                for hh in range(2):
                    h = hps.tile([128, 1024], f32, tag="h", name=f"g{b}_{hh}")
                    for q in range(2):
                        c0 = b * T + hh * 1024 + q * 512
                        nc.tensor.matmul(
                            h[:, q * 512 : (q + 1) * 512],
                            lhsT=rw_r,
                            rhs=xc_t[:, c0 : c0 + 512],
                            start=True,
                            stop=True,
                        )
                    gy = scr.tile([128, 1024], f32, tag="gy", name=f"gy{b}_{hh}")
                    nc.scalar.activation(
                        out=gy,
                        in_=h,
                        func=FT.Gelu,
                        scale=sclA[:, b : b + 1],
                        bias=biaA[:, b : b + 1],
                        accum_out=racc[:, 2 * b + hh : 2 * b + hh + 1],
                    )

            demo_0()
            round1(0)
            demo_1()
            round1(1)
            aggr_front(0)
            demo_2()
            round1(2)
            aggr_back(0)
            demo_3()
            round2(0)
            aggr_front(1)
            round1(3)
            aggr_back(1)
            round2(1)
            aggr_front(2)
            aggr_back(2)
            round2(2)
            aggr_front(3)
            aggr_back(3)
            round2(3)

            # ---- GAP -> r, MLP
            rT = wrk.tile([128, BPC], f32, tag="rT")
            rv = racc.rearrange("p (b two) -> p b two", two=2)
            nc.vector.tensor_add(rT, rv[:, :, 0], rv[:, :, 1])
            nc.vector.tensor_scalar_mul(out=rT, in0=rT, scalar1=1.0 / float(T))

            m1_t = mp_t[:, _M1 : _M1 + HID]
            lnp = mp_t[:, _LNP : _LNP + 3]
            m2_t = mp_t[:, _M2 : _M2 + CTX]
            b2 = mp_t[0:CTX, _B2 : _B2 + 1]
            gw_t = mp_t[0 : CTX + DEMB, _GW : _GW + E]
            gb_t = mp_t[0:E, _GB : _GB + 1]

            nc.tensor.matmul(psm1, lhsT=m1_t, rhs=rT, start=True, stop=True)
            lin = wrk.tile([HID, 2 * BPC], f32, tag="lin")
            nc.vector.tensor_scalar_add(out=lin[:, 0:BPC], in0=psm1, scalar1=lnp[:, 0:1])
            nc.scalar.activation(
                out=lin[:, BPC : 2 * BPC], in_=psm1, func=FT.Square, bias=lnp[:, 0:1]
            )
            nc.tensor.matmul(psls, lhsT=ones_c, rhs=lin, start=True, stop=True)
            lst = wrk.tile([1, 2 * BPC], f32, tag="lst")
            nc.vector.tensor_scalar_mul(out=lst, in0=psls, scalar1=1.0 / float(HID))
            lmsq = wrk.tile([1, BPC], f32, tag="lmsq")
            nc.vector.tensor_mul(lmsq, lst[:, 0:BPC], lst[:, 0:BPC])
            nc.vector.tensor_sub(lst[:, BPC : 2 * BPC], lst[:, BPC : 2 * BPC], lmsq)
            nc.scalar.activation(
                out=lst[:, BPC : 2 * BPC],
                in_=lst[:, BPC : 2 * BPC],
                func=FT.Sqrt,
                bias=eps_c[0:1, :],
            )
            nc.vector.reciprocal(out=lst[:, BPC : 2 * BPC], in_=lst[:, BPC : 2 * BPC])
            nc.tensor.matmul(pslb, lhsT=ones_r, rhs=lst, start=True, stop=True)
            y1 = wrk.tile([HID, BPC], f32, tag="y1")
            nc.vector.tensor_sub(y1, lin[:, 0:BPC], pslb[:, 0:BPC])
            nc.vector.tensor_mul(y1, y1, pslb[:, BPC : 2 * BPC])
            nc.vector.tensor_scalar(
                out=y1,
                in0=y1,
                scalar1=lnp[:, 1:2],
                scalar2=lnp[:, 2:3],
                op0=AL.mult,
                op1=AL.add,
            )
            nc.scalar.activation(out=y1, in_=y1, func=FT.Gelu)
            nc.tensor.matmul(psm2, lhsT=m2_t, rhs=y1, start=True, stop=True)
            nc.vector.tensor_scalar_add(out=catT[0:CTX, :], in0=psm2, scalar1=b2)

            nc.tensor.matmul(psgt, lhsT=gw_t, rhs=catT, start=True, stop=True)
            lg = wrk.tile([E, BPC], f32, tag="lg")
            nc.vector.tensor_scalar_add(out=lg, in0=psgt, scalar1=gb_t)
            dma(out=logout[:, :], in_=lg)

    return _split_multiwait(nc)


# --------------------------------------------------------------------------
# kernel B: shared + 2 selected expert convs, GN+GELU, weighted combine
# --------------------------------------------------------------------------


def _build_b():
    import concourse.bass as bass
    import concourse.tile as tile
    from concourse import mybir

    f32 = mybir.dt.float32
    bf16 = mybir.dt.bfloat16
    FT = mybir.ActivationFunctionType
    AL = mybir.AluOpType

    nc = bass.Bass()
    xcol = nc.dram_tensor("xcol", [KC1, BPC * T], bf16, kind="ExternalInput")
    xct = nc.dram_tensor("xct", [128, BPC * XTW], bf16, kind="ExternalInput")
    wpk = nc.dram_tensor("wpk", [KC1, NC12 * 128], bf16, kind="ExternalInput")
    cst = nc.dram_tensor("cst", [128, 8 + 3 * NC12], f32, kind="ExternalInput")
    giT = nc.dram_tensor("giT", [GROUPS, 128], f32, kind="ExternalInput")
    outd = nc.dram_tensor("out", [BPC, 128, T], f32, kind="ExternalOutput")

    with tile.TileContext(nc) as tc:
        with (
            tc.tile_pool(name="const", bufs=1) as cpool,
            tc.tile_pool(name="wm", bufs=2) as wmp,
            tc.tile_pool(name="agg", bufs=1) as agp,
            tc.tile_pool(name="y", bufs=8) as yp,
            tc.tile_pool(name="tmp", bufs=2) as tp,
            tc.tile_pool(name="ob", bufs=3) as obp,
            tc.tile_pool(name="hps", bufs=3, space="PSUM") as hp,
            tc.tile_pool(name="aps", bufs=2, space="PSUM") as ap2,
        ):
            dma = nc.sync.dma_start
            xc_t = cpool.tile([KC1, BPC * T], bf16)
            xct_t = cpool.tile([128, BPC * XTW], bf16)
            wpk_t = cpool.tile([KC1, NC12 * 128], bf16)
            cst_t = cpool.tile([128, 8 + 3 * NC12], f32)
            giT_t = cpool.tile([GROUPS, 128], f32)
            gsb_t = cpool.tile([KC1, BPC * KC1], bf16)
            ones81 = cpool.tile([KC1, 1], f32)
            eps8 = cpool.tile([GROUPS, 1], f32)

            # DMA order: get batch0's inputs + weights in first
            dma(out=xct_t[:, 0:XTW], in_=xct[:, 0:XTW])
            dma(out=wpk_t, in_=wpk[:, :])
            dma(out=xc_t[:, 0:T], in_=xcol[:, 0:T])
            dma(out=cst_t, in_=cst[:, :])
            dma(out=giT_t, in_=giT[:, :])
            for b in range(1, BPC):
                dma(out=xct_t[:, b * XTW : (b + 1) * XTW], in_=xct[:, b * XTW : (b + 1) * XTW])
                dma(out=xc_t[:, b * T : (b + 1) * T], in_=xcol[:, b * T : (b + 1) * T])
            nc.vector.memset(ones81, 1.0)
            nc.vector.memset(eps8, EPS)

            gi8 = cst_t[:, 0:8]
            gnw = cst_t[:, 8 : 8 + NC12]
            gnb = cst_t[:, 8 + NC12 : 8 + 2 * NC12]
            wv = cst_t[:, 8 + 2 * NC12 : 8 + 3 * NC12]

            ssb = agp.tile([128, 2 * NC12], f32, tag="ssb")
            agg = agp.tile([GROUPS, 12 * BPC], f32, tag="agg")
            bcs = agp.tile([128, 6 * BPC], f32, tag="bcs")
            scl = agp.tile([128, NC12], f32, tag="scl")
            bia = agp.tile([128, NC12], f32, tag="bia")
            tm3 = agp.tile([128, NS * BPC], f32, tag="tm3")

            arenas = {}

            def stats_batch(b):
                arena = ap2.tile([128, 512], f32, tag="arena", name=f"ar{b}")
                arenas[b] = arena
                g_ps = arena[0:KC1, 0:KC1]
                # M1all overlaps G: written only after G's last reader (gsb copy)
                m1_ps = arena[0:KC1, 0 : NS * 128]
                s_ps = arena[:, 392:398]
                gs_ps = arena[0:GROUPS, 400:406]
                bc_ps = arena[:, 408:414]
                # Gram matrix G = xcol_b @ xcol_b^T (accumulated over 16 chunks)
                for ch in range(NCHK):
                    xtc = xct_t[:, b * XTW + ch * KC1 : b * XTW + (ch + 1) * KC1]
                    nc.tensor.matmul(
                        g_ps, lhsT=xtc, rhs=xtc, start=(ch == 0), stop=(ch == NCHK - 1)
                    )
                gsb = gsb_t[:, b * KC1 : (b + 1) * KC1]
                nc.vector.tensor_scalar_mul(out=gsb, in0=g_ps, scalar1=1.0)
                w3 = wpk_t[:, NS * b * 128 : NS * (b + 1) * 128]
                # S1_s = w_s^T G[:,80] (ones-row column = per-row sums)
                for s in range(NS):
                    nc.tensor.matmul(
                        s_ps[:, 2 * s : 2 * s + 1],
                        lhsT=w3[:, s * 128 : (s + 1) * 128],
                        rhs=gsb[:, KC : KC + 1],
                        start=True,
                        stop=True,
                    )
                # M1 = G @ [W0|W1|W2] in one shot, then WM = W*M1, S2 = row-sums
                nc.tensor.matmul(m1_ps, lhsT=gsb, rhs=w3, start=True, stop=True)
                wm = wmp.tile([KC1, NS * 128], f32, tag="wm", name=f"wm{b}")
                nc.vector.tensor_mul(wm, w3, m1_ps)
                for s in range(NS):
                    nc.tensor.matmul(
                        s_ps[:, 2 * s + 1 : 2 * s + 2],
                        lhsT=wm[:, s * 128 : (s + 1) * 128],
                        rhs=ones81,
                        start=True,
                        stop=True,
                    )
                # aggregation -> scl/bia columns [3b:3b+3]
                nc.vector.tensor_scalar_mul(
                    out=ssb[:, 6 * b : 6 * b + 6], in0=s_ps, scalar1=1.0
                )
                nc.tensor.matmul(
                    gs_ps, lhsT=gi8, rhs=ssb[:, 6 * b : 6 * b + 6], start=True, stop=True
                )
                a0 = 12 * b
                mue = agg[:, a0 : a0 + 6]  # [mu|e2] x3
                nc.vector.tensor_scalar_mul(
                    out=mue, in0=gs_ps, scalar1=1.0 / float(GSZ * T)
                )
                muv = mue.rearrange("p (s two) -> p s two", two=2)
                nc.tensor.matmul(
                    bc_ps[:, 0:3], lhsT=giT_t, rhs=muv[:, :, 0], start=True, stop=True
                )
                msq = agg[:, a0 + 6 : a0 + 9]
                nc.vector.tensor_mul(msq, muv[:, :, 0], muv[:, :, 0])
                var = agg[:, a0 + 9 : a0 + 12]
                nc.vector.tensor_sub(var, muv[:, :, 1], msq)
                nc.scalar.activation(out=var, in_=var, func=FT.Sqrt, bias=eps8)
                nc.vector.reciprocal(out=var, in_=var)
                nc.tensor.matmul(
                    bc_ps[:, 3:6], lhsT=giT_t, rhs=var, start=True, stop=True
                )
                nc.vector.tensor_scalar_mul(
                    out=bcs[:, 6 * b : 6 * b + 6], in0=bc_ps, scalar1=1.0
                )
                nc.vector.tensor_mul(
                    scl[:, 3 * b : 3 * b + 3],
                    bcs[:, 6 * b + 3 : 6 * b + 6],
                    gnw[:, 3 * b : 3 * b + 3],
                )
                nc.vector.tensor_mul(
                    tm3[:, 3 * b : 3 * b + 3],
                    bcs[:, 6 * b : 6 * b + 3],
                    scl[:, 3 * b : 3 * b + 3],
                )
                nc.vector.tensor_sub(
                    bia[:, 3 * b : 3 * b + 3],
                    gnb[:, 3 * b : 3 * b + 3],
                    tm3[:, 3 * b : 3 * b + 3],
                )

            def conv_batch(b):
                for hh in range(2):
                    ys = []
                    for s in range(NS):
                        c = NS * b + s
                        h = hp.tile([128, 1024], f32, tag="h", name=f"h{b}_{hh}_{s}")
                        for q in range(2):
                            c0 = b * T + hh * 1024 + q * 512
                            nc.tensor.matmul(
                                h[:, q * 512 : (q + 1) * 512],
                                lhsT=wpk_t[:, c * 128 : (c + 1) * 128],
                                rhs=xc_t[:, c0 : c0 + 512],
                                start=True,
                                stop=True,
                            )
                        y = yp.tile([128, 1024], bf16, tag="y", name=f"y{b}_{hh}_{s}")
                        nc.scalar.activation(
                            out=y,
                            in_=h,
                            func=FT.Gelu,
                            scale=scl[:, c : c + 1],
                            bias=bia[:, c : c + 1],
                        )
                        ys.append(y)
                    nq = 2 if b == BPC - 1 else 1  # chunk the tail batch finer
                    ta = tp.tile([128, 1024], bf16, tag="ta", name=f"ta{b}_{hh}")
                    tb = tp.tile([128, 1024], bf16, tag="tb", name=f"tb{b}_{hh}")
                    sa = tp.tile([128, 1024], bf16, tag="sa", name=f"sa{b}_{hh}")
                    ob = obp.tile([128, 1024], f32, tag="o", name=f"o{b}_{hh}")
                    qw = 1024 // nq
                    for q in range(nq):
                        sl = slice(q * qw, (q + 1) * qw)
                        # DVE: 4x-mode scalar mults, 2x-mode add (all bf16)
                        nc.vector.tensor_scalar_mul(
                            out=ta[:, sl],
                            in0=ys[1][:, sl],
                            scalar1=wv[:, 3 * b + 1 : 3 * b + 2],
                        )
                        nc.vector.tensor_scalar_mul(
                            out=tb[:, sl],
                            in0=ys[2][:, sl],
                            scalar1=wv[:, 3 * b + 2 : 3 * b + 3],
                        )
                        nc.vector.tensor_add(sa[:, sl], ta[:, sl], ys[0][:, sl])
                        # Pool: final add, f32 out
                        nc.gpsimd.tensor_add(ob[:, sl], sa[:, sl], tb[:, sl])
                        nc.sync.dma_start(
                            out=outd[b, :, hh * 1024 + q * qw : hh * 1024 + (q + 1) * qw],
                            in_=ob[:, sl],
                        )

            # staggered emission: stats run one batch ahead of convs
            stats_batch(0)
            stats_batch(1)
            conv_batch(0)
            stats_batch(2)
            conv_batch(1)
            stats_batch(3)
            conv_batch(2)
            conv_batch(3)

    return _split_multiwait(nc)


# --------------------------------------------------------------------------
# host prep
# --------------------------------------------------------------------------


def _im2col(xb):
    # xb: (C, T) f32 -> (81, T) f32, row k*C+c at col t = x[c, t+k-2], row 80 = 1
    col = np.zeros((KC1, T), np.float32)
    for k in range(K):
        lo = max(0, 2 - k)
        hi = min(T, T + 2 - k)
        col[k * C : (k + 1) * C, lo:hi] = xb[:, lo + k - 2 : hi + k - 2]
    col[KC, :] = 1.0
    return col


def _wcol(w):
    # (CO, C, K) -> (80, CO)
    return np.ascontiguousarray(w.transpose(2, 1, 0).reshape(KC, -1)).astype(np.float32)


def _prep_a(inputs):
    f = np.float32
    x = np.asarray(inputs["x"], f)
    demo = np.asarray(inputs["demo"], f)

    rwc = np.zeros((KC1, HID), f)
    rwc[0:KC, :] = _wcol(np.asarray(inputs["rw"], f))

    mlpp = np.zeros((128, _MLPW), f)
    mlpp[0:HID, _M1 : _M1 + HID] = np.asarray(inputs["m1_w"], f).T
    mlpp[0:HID, _LNP + 0] = np.asarray(inputs["m1_b"], f)
    mlpp[0:HID, _LNP + 1] = np.asarray(inputs["ln_g"], f)
    mlpp[0:HID, _LNP + 2] = np.asarray(inputs["ln_b"], f)
    mlpp[0:HID, _M2 : _M2 + CTX] = np.asarray(inputs["m2_w"], f).T
    mlpp[0:CTX, _B2] = np.asarray(inputs["m2_b"], f)
    mlpp[0:DIN, _D1 : _D1 + 2 * DEMB] = np.asarray(inputs["d1_w"], f).T
    mlpp[0 : 2 * DEMB, _DLN + 0] = np.asarray(inputs["d1_b"], f)
    mlpp[0 : 2 * DEMB, _DLN + 1] = np.asarray(inputs["dln_g"], f)
    mlpp[0 : 2 * DEMB, _DLN + 2] = np.asarray(inputs["dln_b"], f)
    mlpp[0 : 2 * DEMB, _D2 : _D2 + DEMB] = np.asarray(inputs["d2_w"], f).T
    mlpp[0:DEMB, _DB2] = np.asarray(inputs["d2_b"], f)
    mlpp[0 : CTX + DEMB, _GW : _GW + E] = np.asarray(inputs["g_w"], f).T
    mlpp[0:E, _GB] = np.asarray(inputs["g_b"], f)
    gind = np.zeros((128, GROUPS), f)
    for ch in range(128):
        gind[ch, ch // GSZ] = 1.0
    mlpp[:, _GI8 : _GI8 + GROUPS] = gind
    mlpp[0:HID, _RGB + 0] = np.asarray(inputs["rg"], f)
    mlpp[0:HID, _RGB + 1] = np.asarray(inputs["rb"], f)
    mlpp[0:GROUPS, _GIT : _GIT + 128] = gind.T

    in_maps = []
    xcols = []  # per core, f32 [81, BPC*T]
    xcts = []   # per core, bf16 [128, BPC*XTW]
    for cid in range(NCORES):
        xcf = np.zeros((KC1, BPC * T), f)
        xct = np.zeros((128, BPC * XTW), BF)
        for b in range(BPC):
            col = _im2col(x[cid * BPC + b])
            xcf[:, b * T : (b + 1) * T] = col
            colb = col.astype(BF)
            xct[:, b * XTW : (b + 1) * XTW] = np.ascontiguousarray(
                colb.T.reshape(NCHK, 128, KC1).transpose(1, 0, 2).reshape(128, XTW)
            )
        xcols.append(xcf)
        xcts.append(xct)
        mlpp_c = mlpp.copy()
        mlpp_c[0:DIN, _DM : _DM + BPC] = demo[cid * BPC : (cid + 1) * BPC].T
        in_maps.append(dict(xcf=xcf, rwc=rwc, mlpp=mlpp_c))
    return in_maps, xcols, xcts, gind


def _prep_b(inputs, logits, xcols, xcts, gind):
    f = np.float32
    sw = np.asarray(inputs["sw"], f)
    sb = np.asarray(inputs["sb"], f)
    sg = np.asarray(inputs["sg"], f)
    sbt = np.asarray(inputs["sbt"], f)
    ew = np.asarray(inputs["ew"], f)
    eb = np.asarray(inputs["eb"], f)
    eg = np.asarray(inputs["eg"], f)
    ebt = np.asarray(inputs["ebt"], f)

    # softmax + top-2 + renormalize (mirrors the reference gate math)
    lm = logits - logits.max(-1, keepdims=True)
    e_ = np.exp(lm, dtype=f)
    ws = e_ / e_.sum(-1, keepdims=True)
    order = np.argsort(-ws, axis=-1, kind="stable")[:, :2]
    w01 = np.take_along_axis(ws, order, axis=-1)
    hard = w01 / (w01.sum(-1, keepdims=True) + f(1e-9))

    in_maps = []
    for cid in range(NCORES):
        wpkc = np.zeros((KC1, NC12 * 128), f)
        cstc = np.zeros((128, 8 + 3 * NC12), f)
        cstc[:, 0:8] = gind
        for b in range(BPC):
            gb = cid * BPC + b
            for s in range(NS):
                c = NS * b + s
                if s == 0:
                    W, cb, gg, bb, wval = sw, sb, sg, sbt, 1.0
                else:
                    ei = int(order[gb, s - 1])
                    W, cb, gg, bb = ew[ei], eb[ei], eg[ei], ebt[ei]
                    wval = float(hard[gb, s - 1])
                wpkc[0:KC, c * 128 : (c + 1) * 128] = _wcol(W)
                wpkc[KC, c * 128 : (c + 1) * 128] = cb
                cstc[:, 8 + c] = gg
                cstc[:, 8 + NC12 + c] = bb
                cstc[:, 8 + 2 * NC12 + c] = wval
        xcol_b = np.zeros((KC1, BPC * T), BF)
        xcol_b[:, :] = xcols[cid].astype(BF)
        in_maps.append(
            dict(
                xcol=xcol_b,
                xct=xcts[cid],
                wpk=wpkc.astype(BF),
                cst=cstc,
                giT=np.ascontiguousarray(gind.T),
            )
        )
    return in_maps


def _run(nc, in_maps, trace=False):
    from concourse.bass_utils import run_bass_kernel_spmd

    return run_bass_kernel_spmd(nc, in_maps, list(range(NCORES)), trace=trace)


def kernel(**inputs):
    import os

    trace = bool(int(os.environ.get("MOE_TRACE", "0")))
    if "a" not in _built:
        _built["a"] = _build_a()
        _built["b"] = _build_b()

    in_a, xcols, xcts, gind = _prep_a(inputs)
    res_a = _run(_built["a"], in_a, trace=trace)
    logits = np.zeros((B, E), np.float32)
    for cid in range(NCORES):
        lt = res_a.results[cid]["logitsT"]  # (E, BPC)
        logits[cid * BPC : (cid + 1) * BPC, :] = lt.T

    in_b = _prep_b(inputs, logits, xcols, xcts, gind)
    res_b = _run(_built["b"], in_b, trace=trace)
    out = np.concatenate([res_b.results[cid]["out"] for cid in range(NCORES)], 0)

    kernel.last_exec_ns = (res_a.exec_time_ns or 0) + (res_b.exec_time_ns or 0)
    kernel.last_results = (res_a, res_b)
    kernel.last_logits = logits
    return out


# revision 28
# speedup vs baseline: 2.5564x; 1.0088x over previous
"""Trainium2 Bass kernel for nn_ContrastiveEncoderMOE.

Strategy: data-parallel over batch (4 batches per core, 8 cores, no
collectives), two launches:

  A) router: im2col conv (f32r, contraction 81 = 5 taps x 16 ch + bias row,
     so each 512-col output tile is ONE matmul) -> GroupNorm with stats
     measured on-device (sum via Pool accum pass + PE matvec, sum-of-squares
     via one-pass DVE tensor_tensor_reduce on the PSUM-resident conv out)
     -> GELU+GAP (ACT accum_out) -> MLP(+LN) -> concat demo embedding ->
     gate logits (per-core (8,4)).
  host: softmax + top-2 + renormalize on (32,8); pack the 2 selected
     experts' conv weights per batch (control plane only).
  B) shared conv + 2 selected expert convs per batch in bf16. GroupNorm
     stats come from the Gram matrix G = xcol @ xcol^T computed on the PE
     from a host-pretransposed copy of the im2col matrix: sum h = w^T G e80
     (ones row), sum h^2 = w^T G w (via G@W on PE, W.M elementwise on DVE,
     ones-matvec on PE). Conv output stays in PSUM; ACT applies the folded
     GroupNorm affine + GELU in one pass; 3-slot weighted combine split
     across Pool (stt1) and DVE (stt2). All DMA issued from the SP engine.
"""

import numpy as np
import ml_dtypes

BF = ml_dtypes.bfloat16

B, C, T = 32, 16, 2048
E, CO, K = 8, 128, 5
HID, CTX, DIN, DEMB = 128, 64, 8, 16
GROUPS = 8
NCORES = 8
BPC = B // NCORES          # batches per core
KC = K * C                 # 80
KC1 = KC + 1               # 81 (bias/ones row)
NS = 3                     # slots: shared, expert0, expert1
NC12 = NS * BPC            # 12
EPS = 1e-5
GSZ = CO // GROUPS         # 16
NCHK = T // 128            # 16 transpose chunks
XTW = NCHK * KC1           # 1296 columns per batch in xcolT

_built = {}


def _split_multiwait(nc, max_waits=1):
    # The pinned walrus rejects >1 sync-wait on one instruction
    # ("Too many sync wait commands"); hoist excess waits onto
    # same-engine NOPs inserted just before.
    from concourse import mybir

    for f in nc.m.functions:
        for blk in f.blocks:
            out = []
            for inst in blk.instructions:
                si = getattr(inst, "sync_info", None)
                if si is not None and si.on_wait and len(si.on_wait) > max_waits:
                    waits = list(si.on_wait)
                    cnt = 0
                    while len(waits) > max_waits:
                        chunk, waits = waits[:max_waits], waits[max_waits:]
                        nop = mybir.InstNoOp(
                            name=f"{inst.name}-mw{cnt}",
                            engine=inst.engine,
                            bass_nofuse=True,
                            sync_info=mybir.SyncInfo(on_wait=chunk, on_update=[]),
                        )
                        out.append(nop)
                        cnt += 1
                    inst.sync_info = mybir.SyncInfo(
                        on_wait=waits, on_update=list(si.on_update)
                    )
                out.append(inst)
            blk.instructions[:] = out
    return nc


# --------------------------------------------------------------------------
# kernel A: router -> gate logits
# --------------------------------------------------------------------------

# packed const layout (columns of the [128, 399] f32 "mlpp" tensor)
_M1 = 0          # [128, 0:128]   m1_w.T
_LNP = 128       # [128, 128:131] m1_b | ln_g | ln_b
_M2 = 131        # [128, 131:195] m2_w.T
_B2 = 195        # [64, 195:196]
_DM = 196        # [8, 196:200]   demo.T
_D1 = 200        # [8, 200:232]   d1_w.T
_DLN = 232       # [32, 232:235]  d1_b | dln_g | dln_b
_D2 = 235        # [32, 235:251]  d2_w.T
_DB2 = 251       # [16, 251:252]
_GW = 252        # [80, 252:260]  g_w.T
_GB = 260        # [8, 260:261]
_GI8 = 261       # [128, 261:269] one-hot group matrix
_RGB = 269       # [128, 269:271] rg | rb
_GIT = 271       # [8, 271:399]   one-hot group matrix transposed
_MLPW = 399


def _build_a():
    import concourse.bass as bass
    import concourse.tile as tile
    from concourse import mybir

    f32 = mybir.dt.float32
    f32r = mybir.dt.float32r
    FT = mybir.ActivationFunctionType
    AL = mybir.AluOpType

    nc = bass.Bass()
    xcf = nc.dram_tensor("xcf", [KC1, BPC * T], f32r, kind="ExternalInput")
    rwc = nc.dram_tensor("rwc", [KC1, HID], f32r, kind="ExternalInput")
    mlpp = nc.dram_tensor("mlpp", [128, _MLPW], f32, kind="ExternalInput")
    logout = nc.dram_tensor("logitsT", [E, BPC], f32, kind="ExternalOutput")

    with tile.TileContext(nc) as tc:
        with (
            tc.tile_pool(name="const", bufs=1) as cst,
            tc.tile_pool(name="work", bufs=1) as wrk,
            tc.tile_pool(name="scr", bufs=4) as scr,
            tc.tile_pool(name="hps", bufs=3, space="PSUM") as hps,
            tc.tile_pool(name="aps", bufs=1, space="PSUM") as aps,
        ):
            dma = nc.sync.dma_start
            xc_t = cst.tile([KC1, BPC * T], f32r)
            rw_t = cst.tile([KC1, HID], f32r)
            mp_t = cst.tile([128, _MLPW], f32)
            dma(out=xc_t[:, 0:512], in_=xcf[:, 0:512])
            dma(out=mp_t[:, _DM:_GB], in_=mlpp[:, _DM:_GB])  # demo params early
            dma(out=rw_t, in_=rwc[:, :])
            dma(out=xc_t[:, 512:1024], in_=xcf[:, 512:1024])
            dma(out=xc_t[:, 1024:T], in_=xcf[:, 1024:T])
            dma(out=mp_t[:, 0:_DM], in_=mlpp[:, 0:_DM])
            dma(out=mp_t[:, _GB:_MLPW], in_=mlpp[:, _GB:_MLPW])
            for b in range(1, BPC):
                dma(out=xc_t[:, b * T : (b + 1) * T], in_=xcf[:, b * T : (b + 1) * T])

            ones_c = cst.tile([128, 1], f32)
            nc.vector.memset(ones_c, 1.0)
            ones_r = cst.tile([1, 128], f32)
            nc.vector.memset(ones_r, 1.0)
            eps_c = cst.tile([128, 1], f32)
            nc.vector.memset(eps_c, EPS)

            # PSUM arena (one bank), hand-sliced
            arena = aps.tile([128, 512], f32, tag="arena")
            s_ps = arena[:, 0:4]         # batched S1 matvec out [128, BPC]
            gs_ps = arena[0:8, 8:20]     # per-batch [8,3] at 8+3b
            bc_ps = arena[:, 20:28]      # per-batch [128,2] at 20+2b
            psm1 = arena[:, 28:32]
            psls = arena[0:1, 32:40]
            pslb = arena[:, 40:48]
            psm2 = arena[0:CTX, 48:52]
            psgt = arena[0:E, 52:56]
            psd1 = arena[0 : 2 * DEMB, 56:60]
            psds = arena[0:1, 60:68]
            psdb = arena[0 : 2 * DEMB, 68:76]
            psd2 = arena[0:DEMB, 76:80]

            # ---- demo path (emitted mid-stream once mlpp has landed)
            dm_t = mp_t[0:DIN, _DM : _DM + BPC]
            d1_t = mp_t[0:DIN, _D1 : _D1 + 2 * DEMB]
            dlnp = mp_t[0 : 2 * DEMB, _DLN : _DLN + 3]
            d2_t = mp_t[0 : 2 * DEMB, _D2 : _D2 + DEMB]
            db2 = mp_t[0:DEMB, _DB2 : _DB2 + 1]
            catT = wrk.tile([CTX + DEMB, BPC], f32, tag="cat")

            dln = wrk.tile([2 * DEMB, 2 * BPC], f32, tag="dln")
            dst = wrk.tile([1, 2 * BPC], f32, tag="dst")

            def demo_0():
              nc.tensor.matmul(psd1, lhsT=d1_t, rhs=dm_t, start=True, stop=True)

            def demo_1():
              nc.vector.tensor_scalar_add(
                  out=dln[:, 0:BPC], in0=psd1, scalar1=dlnp[:, 0:1]
              )
              nc.scalar.activation(
                  out=dln[:, BPC : 2 * BPC],
                  in_=psd1,
                  func=FT.Square,
                  bias=dlnp[:, 0:1],
              )
              nc.tensor.matmul(
                  psds, lhsT=ones_c[0 : 2 * DEMB, :], rhs=dln, start=True, stop=True
              )

            def demo_2():
              nc.vector.tensor_scalar_mul(out=dst, in0=psds, scalar1=1.0 / (2 * DEMB))
              dmsq = wrk.tile([1, BPC], f32, tag="dmsq")
              nc.vector.tensor_mul(dmsq, dst[:, 0:BPC], dst[:, 0:BPC])
              nc.vector.tensor_sub(dst[:, BPC : 2 * BPC], dst[:, BPC : 2 * BPC], dmsq)
              nc.scalar.activation(
                  out=dst[:, BPC : 2 * BPC],
                  in_=dst[:, BPC : 2 * BPC],
                  func=FT.Sqrt,
                  bias=eps_c[0:1, :],
              )
              nc.vector.reciprocal(
                  out=dst[:, BPC : 2 * BPC], in_=dst[:, BPC : 2 * BPC]
              )
              nc.tensor.matmul(
                  psdb, lhsT=ones_r[:, 0 : 2 * DEMB], rhs=dst, start=True, stop=True
              )

            def demo_3():
              dy = wrk.tile([2 * DEMB, BPC], f32, tag="dy")
              nc.vector.tensor_sub(dy, dln[:, 0:BPC], psdb[:, 0:BPC])
              nc.vector.tensor_mul(dy, dy, psdb[:, BPC : 2 * BPC])
              nc.vector.tensor_scalar(
                  out=dy,
                  in0=dy,
                  scalar1=dlnp[:, 1:2],
                  scalar2=dlnp[:, 2:3],
                  op0=AL.mult,
                  op1=AL.add,
              )
              nc.scalar.activation(out=dy, in_=dy, func=FT.Gelu)
              nc.tensor.matmul(psd2, lhsT=d2_t, rhs=dy, start=True, stop=True)
              nc.vector.tensor_scalar_add(
                  out=catT[CTX : CTX + DEMB, :], in0=psd2, scalar1=db2
              )

            # ---- router conv + GN + GELU + GAP, per batch
            gi8 = mp_t[:, _GI8 : _GI8 + GROUPS]
            giT8 = mp_t[0:GROUPS, _GIT : _GIT + 128]
            rgb = mp_t[:, _RGB : _RGB + 2]
            agg = wrk.tile([GROUPS, 8 * BPC], f32, tag="agg")
            bcs = wrk.tile([128, 2 * BPC], f32, tag="bcs")
            sclA = wrk.tile([128, BPC], f32, tag="sclA")
            biaA = wrk.tile([128, BPC], f32, tag="biaA")
            tmpA = wrk.tile([128, BPC], f32, tag="tmpA")
            racc = wrk.tile([128, 2 * BPC], f32, tag="racc")
            bnst = wrk.tile([128, BPC, 4, 6], f32, tag="bnst")
            mvs = wrk.tile([128, BPC, 2], f32, tag="mvs")
            e2c = wrk.tile([128, BPC], f32, tag="e2c")
            tmv = wrk.tile([128, BPC], f32, tag="tmv")
            rw_r = rw_t[:, :]

            def round1(b):
                # conv halves -> bn_stats; tiles freed after stats
                for hh in range(2):
                    h = hps.tile([128, 1024], f32, tag="h", name=f"h{b}_{hh}")
                    for q in range(2):
                        c0 = b * T + hh * 1024 + q * 512
                        nc.tensor.matmul(
                            h[:, q * 512 : (q + 1) * 512],
                            lhsT=rw_r,
                            rhs=xc_t[:, c0 : c0 + 512],
                            start=True,
                            stop=True,
                        )
                    for q in range(2):
                        nc.vector.bn_stats(
                            out=bnst[:, b, 2 * hh + q, :],
                            in_=h[:, q * 512 : (q + 1) * 512],
                        )

            def aggr_front(b):
                # per-channel mean/var -> E[h], E[h^2] columns -> group reduce
                nc.vector.bn_aggr(out=mvs[:, b, :], in_=bnst[:, b])
                nc.vector.tensor_mul(tmv[:, b : b + 1], mvs[:, b, 0:1], mvs[:, b, 0:1])
                nc.vector.tensor_add(e2c[:, b : b + 1], mvs[:, b, 1:2], tmv[:, b : b + 1])
                gsb = gs_ps[:, 3 * b : 3 * b + 2]
                nc.tensor.matmul(
                    gsb[:, 0:1], lhsT=gi8, rhs=mvs[:, b, 0:1], start=True, stop=True
                )
                nc.tensor.matmul(
                    gsb[:, 1:2], lhsT=gi8, rhs=e2c[:, b : b + 1], start=True, stop=True
                )
                a0 = 8 * b
                gss = agg[:, a0 : a0 + 2]  # mu | e2 (group means)
                nc.vector.tensor_scalar_mul(out=gss, in0=gsb, scalar1=1.0 / float(GSZ))
                msq = agg[:, a0 + 4 : a0 + 5]
                nc.vector.tensor_mul(msq, gss[:, 0:1], gss[:, 0:1])
                var = agg[:, a0 + 5 : a0 + 6]
                nc.vector.tensor_sub(var, gss[:, 1:2], msq)
                nc.scalar.activation(
                    out=var, in_=var, func=FT.Sqrt, bias=eps_c[0:GROUPS, :]
                )
                nc.vector.reciprocal(out=var, in_=var)

            def aggr_back(b):
                a0 = 8 * b
                gss = agg[:, a0 : a0 + 2]
                var = agg[:, a0 + 5 : a0 + 6]
                bcb = bc_ps[:, 2 * b : 2 * b + 2]
                nc.tensor.matmul(
                    bcb[:, 0:1], lhsT=giT8, rhs=gss[:, 0:1], start=True, stop=True
                )
                nc.tensor.matmul(
                    bcb[:, 1:2], lhsT=giT8, rhs=var, start=True, stop=True
                )
                nc.vector.tensor_scalar_mul(
                    out=bcs[:, 2 * b : 2 * b + 2], in0=bcb, scalar1=1.0
                )
                nc.vector.tensor_mul(
                    sclA[:, b : b + 1], bcs[:, 2 * b + 1 : 2 * b + 2], rgb[:, 0:1]
                )
                nc.vector.tensor_mul(
                    tmpA[:, b : b + 1], bcs[:, 2 * b : 2 * b + 1], sclA[:, b : b + 1]
                )
                nc.vector.tensor_sub(biaA[:, b : b + 1], rgb[:, 1:2], tmpA[:, b : b + 1])

            def round2(b):
                # recompute conv, then GELU + GAP accumulate
                for hh in range(2):
                    h = hps.tile([128, 1024], f32, tag="h", name=f"g{b}_{hh}")
                    for q in range(2):
                        c0 = b * T + hh * 1024 + q * 512
                        nc.tensor.matmul(
                            h[:, q * 512 : (q + 1) * 512],
                            lhsT=rw_r,
                            rhs=xc_t[:, c0 : c0 + 512],
                            start=True,
                            stop=True,
                        )
                    gy = scr.tile([128, 1024], f32, tag="gy", name=f"gy{b}_{hh}")
                    nc.scalar.activation(
                        out=gy,
                        in_=h,
                        func=FT.Gelu,
                        scale=sclA[:, b : b + 1],
                        bias=biaA[:, b : b + 1],
                        accum_out=racc[:, 2 * b + hh : 2 * b + hh + 1],
                    )

            demo_0()
            round1(0)
            demo_1()
            round1(1)
            aggr_front(0)
            demo_2()
            round1(2)
            aggr_back(0)
            demo_3()
            round2(0)
            aggr_front(1)
            round1(3)
            aggr_back(1)
            round2(1)
            aggr_front(2)
            aggr_back(2)
            round2(2)
            aggr_front(3)
            aggr_back(3)
            round2(3)

            # ---- GAP -> r, MLP
            rT = wrk.tile([128, BPC], f32, tag="rT")
            rv = racc.rearrange("p (b two) -> p b two", two=2)
            nc.vector.tensor_add(rT, rv[:, :, 0], rv[:, :, 1])
            nc.vector.tensor_scalar_mul(out=rT, in0=rT, scalar1=1.0 / float(T))

            m1_t = mp_t[:, _M1 : _M1 + HID]
            lnp = mp_t[:, _LNP : _LNP + 3]
            m2_t = mp_t[:, _M2 : _M2 + CTX]
            b2 = mp_t[0:CTX, _B2 : _B2 + 1]
            gw_t = mp_t[0 : CTX + DEMB, _GW : _GW + E]
            gb_t = mp_t[0:E, _GB : _GB + 1]

            nc.tensor.matmul(psm1, lhsT=m1_t, rhs=rT, start=True, stop=True)
            lin = wrk.tile([HID, 2 * BPC], f32, tag="lin")
            nc.vector.tensor_scalar_add(out=lin[:, 0:BPC], in0=psm1, scalar1=lnp[:, 0:1])
            nc.scalar.activation(
                out=lin[:, BPC : 2 * BPC], in_=psm1, func=FT.Square, bias=lnp[:, 0:1]
            )
            nc.tensor.matmul(psls, lhsT=ones_c, rhs=lin, start=True, stop=True)
            lst = wrk.tile([1, 2 * BPC], f32, tag="lst")
            nc.vector.tensor_scalar_mul(out=lst, in0=psls, scalar1=1.0 / float(HID))
            lmsq = wrk.tile([1, BPC], f32, tag="lmsq")
            nc.vector.tensor_mul(lmsq, lst[:, 0:BPC], lst[:, 0:BPC])
            nc.vector.tensor_sub(lst[:, BPC : 2 * BPC], lst[:, BPC : 2 * BPC], lmsq)
            nc.scalar.activation(
                out=lst[:, BPC : 2 * BPC],
                in_=lst[:, BPC : 2 * BPC],
                func=FT.Sqrt,
                bias=eps_c[0:1, :],
            )
            nc.vector.reciprocal(out=lst[:, BPC : 2 * BPC], in_=lst[:, BPC : 2 * BPC])
            nc.tensor.matmul(pslb, lhsT=ones_r, rhs=lst, start=True, stop=True)
            y1 = wrk.tile([HID, BPC], f32, tag="y1")
            nc.vector.tensor_sub(y1, lin[:, 0:BPC], pslb[:, 0:BPC])
            nc.vector.tensor_mul(y1, y1, pslb[:, BPC : 2 * BPC])
            nc.vector.tensor_scalar(
                out=y1,
                in0=y1,
                scalar1=lnp[:, 1:2],
                scalar2=lnp[:, 2:3],
                op0=AL.mult,
                op1=AL.add,
            )
            nc.scalar.activation(out=y1, in_=y1, func=FT.Gelu)
            nc.tensor.matmul(psm2, lhsT=m2_t, rhs=y1, start=True, stop=True)
            nc.vector.tensor_scalar_add(out=catT[0:CTX, :], in0=psm2, scalar1=b2)

            nc.tensor.matmul(psgt, lhsT=gw_t, rhs=catT, start=True, stop=True)
            lg = wrk.tile([E, BPC], f32, tag="lg")
            nc.vector.tensor_scalar_add(out=lg, in0=psgt, scalar1=gb_t)
            dma(out=logout[:, :], in_=lg)

    return _split_multiwait(nc)


# --------------------------------------------------------------------------
# kernel B: shared + 2 selected expert convs, GN+GELU, weighted combine
# --------------------------------------------------------------------------


def _build_b():
    import concourse.bass as bass
    import concourse.tile as tile
    from concourse import mybir

    f32 = mybir.dt.float32
    bf16 = mybir.dt.bfloat16
    FT = mybir.ActivationFunctionType
    AL = mybir.AluOpType

    nc = bass.Bass()
    xcol = nc.dram_tensor("xcol", [KC1, BPC * T], bf16, kind="ExternalInput")
    xct = nc.dram_tensor("xct", [128, BPC * XTW], bf16, kind="ExternalInput")
    wpk = nc.dram_tensor("wpk", [KC1, NC12 * 128], bf16, kind="ExternalInput")
    cst = nc.dram_tensor("cst", [128, 8 + 3 * NC12], f32, kind="ExternalInput")
    giT = nc.dram_tensor("giT", [GROUPS, 128], f32, kind="ExternalInput")
    outd = nc.dram_tensor("out", [BPC, 128, T], f32, kind="ExternalOutput")

    with tile.TileContext(nc) as tc:
        with (
            tc.tile_pool(name="const", bufs=1) as cpool,
            tc.tile_pool(name="wm", bufs=2) as wmp,
            tc.tile_pool(name="agg", bufs=1) as agp,
            tc.tile_pool(name="y", bufs=8) as yp,
            tc.tile_pool(name="tmp", bufs=2) as tp,
            tc.tile_pool(name="ob", bufs=3) as obp,
            tc.tile_pool(name="hps", bufs=3, space="PSUM") as hp,
            tc.tile_pool(name="aps", bufs=2, space="PSUM") as ap2,
        ):
            dma = nc.sync.dma_start
            xc_t = cpool.tile([KC1, BPC * T], bf16)
            xct_t = cpool.tile([128, BPC * XTW], bf16)
            wpk_t = cpool.tile([KC1, NC12 * 128], bf16)
            cst_t = cpool.tile([128, 8 + 3 * NC12], f32)
            giT_t = cpool.tile([GROUPS, 128], f32)
            gsb_t = cpool.tile([KC1, BPC * KC1], bf16)
            ones81 = cpool.tile([KC1, 1], f32)
            eps8 = cpool.tile([GROUPS, 1], f32)

            # DMA order: get batch0's inputs + weights in first
            dma(out=xct_t[:, 0:XTW], in_=xct[:, 0:XTW])
            dma(out=wpk_t, in_=wpk[:, :])
            dma(out=xc_t[:, 0:T], in_=xcol[:, 0:T])
            dma(out=cst_t, in_=cst[:, :])
            dma(out=giT_t, in_=giT[:, :])
            for b in range(1, BPC):
                dma(out=xct_t[:, b * XTW : (b + 1) * XTW], in_=xct[:, b * XTW : (b + 1) * XTW])
                dma(out=xc_t[:, b * T : (b + 1) * T], in_=xcol[:, b * T : (b + 1) * T])
            nc.vector.memset(ones81, 1.0)
            nc.vector.memset(eps8, EPS)

            gi8 = cst_t[:, 0:8]
            gnw = cst_t[:, 8 : 8 + NC12]
            gnb = cst_t[:, 8 + NC12 : 8 + 2 * NC12]
            wv = cst_t[:, 8 + 2 * NC12 : 8 + 3 * NC12]

            ssb = agp.tile([128, 2 * NC12], f32, tag="ssb")
            agg = agp.tile([GROUPS, 12 * BPC], f32, tag="agg")
            bcs = agp.tile([128, 6 * BPC], f32, tag="bcs")
            scl = agp.tile([128, NC12], f32, tag="scl")
            bia = agp.tile([128, NC12], f32, tag="bia")
            tm3 = agp.tile([128, NS * BPC], f32, tag="tm3")

            arenas = {}

            def stats_batch(b):
                arena = ap2.tile([128, 512], f32, tag="arena", name=f"ar{b}")
                arenas[b] = arena
                g_ps = arena[0:KC1, 0:KC1]
                # M1all overlaps G: written only after G's last reader (gsb copy)
                m1_ps = arena[0:KC1, 0 : NS * 128]
                s_ps = arena[:, 392:398]
                gs_ps = arena[0:GROUPS, 400:406]
                bc_ps = arena[:, 408:414]
                # Gram matrix G = xcol_b @ xcol_b^T (accumulated over 16 chunks)
                for ch in range(NCHK):
                    xtc = xct_t[:, b * XTW + ch * KC1 : b * XTW + (ch + 1) * KC1]
                    nc.tensor.matmul(
                        g_ps, lhsT=xtc, rhs=xtc, start=(ch == 0), stop=(ch == NCHK - 1)
                    )
                gsb = gsb_t[:, b * KC1 : (b + 1) * KC1]
                nc.vector.tensor_scalar_mul(out=gsb, in0=g_ps, scalar1=1.0)
                w3 = wpk_t[:, NS * b * 128 : NS * (b + 1) * 128]
                # S1_s = w_s^T G[:,80] (ones-row column = per-row sums)
                for s in range(NS):
                    nc.tensor.matmul(
                        s_ps[:, 2 * s : 2 * s + 1],
                        lhsT=w3[:, s * 128 : (s + 1) * 128],
                        rhs=gsb[:, KC : KC + 1],
                        start=True,
                        stop=True,
                    )
                # M1 = G @ [W0|W1|W2] in one shot, then WM = W*M1, S2 = row-sums
                nc.tensor.matmul(m1_ps, lhsT=gsb, rhs=w3, start=True, stop=True)
                wm = wmp.tile([KC1, NS * 128], f32, tag="wm", name=f"wm{b}")
                nc.vector.tensor_mul(wm, w3, m1_ps)
                for s in range(NS):
                    nc.tensor.matmul(
                        s_ps[:, 2 * s + 1 : 2 * s + 2],
                        lhsT=wm[:, s * 128 : (s + 1) * 128],
                        rhs=ones81,
                        start=True,
                        stop=True,
                    )
                # aggregation -> scl/bia columns [3b:3b+3]
                nc.vector.tensor_scalar_mul(
                    out=ssb[:, 6 * b : 6 * b + 6], in0=s_ps, scalar1=1.0
                )
                nc.tensor.matmul(
                    gs_ps, lhsT=gi8, rhs=ssb[:, 6 * b : 6 * b + 6], start=True, stop=True
                )
                a0 = 12 * b
                mue = agg[:, a0 : a0 + 6]  # [mu|e2] x3
                nc.vector.tensor_scalar_mul(
                    out=mue, in0=gs_ps, scalar1=1.0 / float(GSZ * T)
                )
                muv = mue.rearrange("p (s two) -> p s two", two=2)
                nc.tensor.matmul(
                    bc_ps[:, 0:3], lhsT=giT_t, rhs=muv[:, :, 0], start=True, stop=True
                )
                msq = agg[:, a0 + 6 : a0 + 9]
                nc.vector.tensor_mul(msq, muv[:, :, 0], muv[:, :, 0])
                var = agg[:, a0 + 9 : a0 + 12]
                nc.vector.tensor_sub(var, muv[:, :, 1], msq)
                nc.scalar.activation(out=var, in_=var, func=FT.Sqrt, bias=eps8)
                nc.vector.reciprocal(out=var, in_=var)
                nc.tensor.matmul(
                    bc_ps[:, 3:6], lhsT=giT_t, rhs=var, start=True, stop=True
                )
                nc.vector.tensor_scalar_mul(
                    out=bcs[:, 6 * b : 6 * b + 6], in0=bc_ps, scalar1=1.0
                )
                nc.vector.tensor_mul(
                    scl[:, 3 * b : 3 * b + 3],
                    bcs[:, 6 * b + 3 : 6 * b + 6],
                    gnw[:, 3 * b : 3 * b + 3],
                )
                nc.vector.tensor_mul(
                    tm3[:, 3 * b : 3 * b + 3],
                    bcs[:, 6 * b : 6 * b + 3],
                    scl[:, 3 * b : 3 * b + 3],
                )
                nc.vector.tensor_sub(
                    bia[:, 3 * b : 3 * b + 3],
                    gnb[:, 3 * b : 3 * b + 3],
                    tm3[:, 3 * b : 3 * b + 3],
                )

            def conv_batch(b):
                for hh in range(2):
                    ys = []
                    for s in range(NS):
                        c = NS * b + s
                        h = hp.tile([128, 1024], f32, tag="h", name=f"h{b}_{hh}_{s}")
                        for q in range(2):
                            c0 = b * T + hh * 1024 + q * 512
                            nc.tensor.matmul(
                                h[:, q * 512 : (q + 1) * 512],
                                lhsT=wpk_t[:, c * 128 : (c + 1) * 128],
                                rhs=xc_t[:, c0 : c0 + 512],
                                start=True,
                                stop=True,
                            )
                        y = yp.tile([128, 1024], bf16, tag="y", name=f"y{b}_{hh}_{s}")
                        nc.scalar.activation(
                            out=y,
                            in_=h,
                            func=FT.Gelu,
                            scale=scl[:, c : c + 1],
                            bias=bia[:, c : c + 1],
                        )
                        ys.append(y)
                    nq = 2 if b == BPC - 1 else 1  # chunk the tail batch finer
                    ta = tp.tile([128, 1024], bf16, tag="ta", name=f"ta{b}_{hh}")
                    tb = tp.tile([128, 1024], bf16, tag="tb", name=f"tb{b}_{hh}")
                    sa = tp.tile([128, 1024], bf16, tag="sa", name=f"sa{b}_{hh}")
                    ob = obp.tile([128, 1024], f32, tag="o", name=f"o{b}_{hh}")
                    qw = 1024 // nq
                    for q in range(nq):
                        sl = slice(q * qw, (q + 1) * qw)
                        # DVE: 4x-mode scalar mults, 2x-mode add (all bf16)
                        nc.vector.tensor_scalar_mul(
                            out=ta[:, sl],
                            in0=ys[1][:, sl],
                            scalar1=wv[:, 3 * b + 1 : 3 * b + 2],
                        )
                        nc.vector.tensor_scalar_mul(
                            out=tb[:, sl],
                            in0=ys[2][:, sl],
                            scalar1=wv[:, 3 * b + 2 : 3 * b + 3],
                        )
                        nc.vector.tensor_add(sa[:, sl], ta[:, sl], ys[0][:, sl])
                        # final add (f32 out): Pool in steady state, DVE on the
                        # tail batch where the Pool op's latency gates the end
                        if b == BPC - 1:
                            nc.vector.tensor_add(ob[:, sl], sa[:, sl], tb[:, sl])
                        else:
                            nc.gpsimd.tensor_add(ob[:, sl], sa[:, sl], tb[:, sl])
                        nc.sync.dma_start(
                            out=outd[b, :, hh * 1024 + q * qw : hh * 1024 + (q + 1) * qw],
                            in_=ob[:, sl],
                        )

            # staggered emission: stats run one batch ahead of convs
            stats_batch(0)
            stats_batch(1)
            conv_batch(0)
            stats_batch(2)
            conv_batch(1)
            stats_batch(3)
            conv_batch(2)
            conv_batch(3)

    return _split_multiwait(nc)


# --------------------------------------------------------------------------
# host prep
# --------------------------------------------------------------------------


def _im2col(xb):
    # xb: (C, T) f32 -> (81, T) f32, row k*C+c at col t = x[c, t+k-2], row 80 = 1
    col = np.zeros((KC1, T), np.float32)
    for k in range(K):
        lo = max(0, 2 - k)
        hi = min(T, T + 2 - k)
        col[k * C : (k + 1) * C, lo:hi] = xb[:, lo + k - 2 : hi + k - 2]
    col[KC, :] = 1.0
    return col


def _wcol(w):
    # (CO, C, K) -> (80, CO)
    return np.ascontiguousarray(w.transpose(2, 1, 0).reshape(KC, -1)).astype(np.float32)


def _prep_a(inputs):
    f = np.float32
    x = np.asarray(inputs["x"], f)
    demo = np.asarray(inputs["demo"], f)

    rwc = np.zeros((KC1, HID), f)
    rwc[0:KC, :] = _wcol(np.asarray(inputs["rw"], f))

    mlpp = np.zeros((128, _MLPW), f)
    mlpp[0:HID, _M1 : _M1 + HID] = np.asarray(inputs["m1_w"], f).T
    mlpp[0:HID, _LNP + 0] = np.asarray(inputs["m1_b"], f)
    mlpp[0:HID, _LNP + 1] = np.asarray(inputs["ln_g"], f)
    mlpp[0:HID, _LNP + 2] = np.asarray(inputs["ln_b"], f)
    mlpp[0:HID, _M2 : _M2 + CTX] = np.asarray(inputs["m2_w"], f).T
    mlpp[0:CTX, _B2] = np.asarray(inputs["m2_b"], f)
    mlpp[0:DIN, _D1 : _D1 + 2 * DEMB] = np.asarray(inputs["d1_w"], f).T
    mlpp[0 : 2 * DEMB, _DLN + 0] = np.asarray(inputs["d1_b"], f)
    mlpp[0 : 2 * DEMB, _DLN + 1] = np.asarray(inputs["dln_g"], f)
    mlpp[0 : 2 * DEMB, _DLN + 2] = np.asarray(inputs["dln_b"], f)
    mlpp[0 : 2 * DEMB, _D2 : _D2 + DEMB] = np.asarray(inputs["d2_w"], f).T
    mlpp[0:DEMB, _DB2] = np.asarray(inputs["d2_b"], f)
    mlpp[0 : CTX + DEMB, _GW : _GW + E] = np.asarray(inputs["g_w"], f).T
    mlpp[0:E, _GB] = np.asarray(inputs["g_b"], f)
    gind = np.zeros((128, GROUPS), f)
    for ch in range(128):
        gind[ch, ch // GSZ] = 1.0
    mlpp[:, _GI8 : _GI8 + GROUPS] = gind
    mlpp[0:HID, _RGB + 0] = np.asarray(inputs["rg"], f)
    mlpp[0:HID, _RGB + 1] = np.asarray(inputs["rb"], f)
    mlpp[0:GROUPS, _GIT : _GIT + 128] = gind.T

    in_maps = []
    xcols = []  # per core, f32 [81, BPC*T]
    xcts = []   # per core, bf16 [128, BPC*XTW]
    for cid in range(NCORES):
        xcf = np.zeros((KC1, BPC * T), f)
        xct = np.zeros((128, BPC * XTW), BF)
        for b in range(BPC):
            col = _im2col(x[cid * BPC + b])
            xcf[:, b * T : (b + 1) * T] = col
            colb = col.astype(BF)
            xct[:, b * XTW : (b + 1) * XTW] = np.ascontiguousarray(
                colb.T.reshape(NCHK, 128, KC1).transpose(1, 0, 2).reshape(128, XTW)
            )
        xcols.append(xcf)
        xcts.append(xct)
        mlpp_c = mlpp.copy()
        mlpp_c[0:DIN, _DM : _DM + BPC] = demo[cid * BPC : (cid + 1) * BPC].T
        in_maps.append(dict(xcf=xcf, rwc=rwc, mlpp=mlpp_c))
    return in_maps, xcols, xcts, gind


def _prep_b(inputs, logits, xcols, xcts, gind):
    f = np.float32
    sw = np.asarray(inputs["sw"], f)
    sb = np.asarray(inputs["sb"], f)
    sg = np.asarray(inputs["sg"], f)
    sbt = np.asarray(inputs["sbt"], f)
    ew = np.asarray(inputs["ew"], f)
    eb = np.asarray(inputs["eb"], f)
    eg = np.asarray(inputs["eg"], f)
    ebt = np.asarray(inputs["ebt"], f)

    # softmax + top-2 + renormalize (mirrors the reference gate math)
    lm = logits - logits.max(-1, keepdims=True)
    e_ = np.exp(lm, dtype=f)
    ws = e_ / e_.sum(-1, keepdims=True)
    order = np.argsort(-ws, axis=-1, kind="stable")[:, :2]
    w01 = np.take_along_axis(ws, order, axis=-1)
    hard = w01 / (w01.sum(-1, keepdims=True) + f(1e-9))

    in_maps = []
    for cid in range(NCORES):
        wpkc = np.zeros((KC1, NC12 * 128), f)
        cstc = np.zeros((128, 8 + 3 * NC12), f)
        cstc[:, 0:8] = gind
        for b in range(BPC):
            gb = cid * BPC + b
            for s in range(NS):
                c = NS * b + s
                if s == 0:
                    W, cb, gg, bb, wval = sw, sb, sg, sbt, 1.0
                else:
                    ei = int(order[gb, s - 1])
                    W, cb, gg, bb = ew[ei], eb[ei], eg[ei], ebt[ei]
                    wval = float(hard[gb, s - 1])
                wpkc[0:KC, c * 128 : (c + 1) * 128] = _wcol(W)
                wpkc[KC, c * 128 : (c + 1) * 128] = cb
                cstc[:, 8 + c] = gg
                cstc[:, 8 + NC12 + c] = bb
                cstc[:, 8 + 2 * NC12 + c] = wval
        xcol_b = np.zeros((KC1, BPC * T), BF)
        xcol_b[:, :] = xcols[cid].astype(BF)
        in_maps.append(
            dict(
                xcol=xcol_b,
                xct=xcts[cid],
                wpk=wpkc.astype(BF),
                cst=cstc,
                giT=np.ascontiguousarray(gind.T),
            )
        )
    return in_maps


def _run(nc, in_maps, trace=False):
    from concourse.bass_utils import run_bass_kernel_spmd

    return run_bass_kernel_spmd(nc, in_maps, list(range(NCORES)), trace=trace)


def kernel(**inputs):
    import os

    trace = bool(int(os.environ.get("MOE_TRACE", "0")))
    if "a" not in _built:
        _built["a"] = _build_a()
        _built["b"] = _build_b()

    in_a, xcols, xcts, gind = _prep_a(inputs)
    res_a = _run(_built["a"], in_a, trace=trace)
    logits = np.zeros((B, E), np.float32)
    for cid in range(NCORES):
        lt = res_a.results[cid]["logitsT"]  # (E, BPC)
        logits[cid * BPC : (cid + 1) * BPC, :] = lt.T

    in_b = _prep_b(inputs, logits, xcols, xcts, gind)
    res_b = _run(_built["b"], in_b, trace=trace)
    out = np.concatenate([res_b.results[cid]["out"] for cid in range(NCORES)], 0)

    kernel.last_exec_ns = (res_a.exec_time_ns or 0) + (res_b.exec_time_ns or 0)
    kernel.last_results = (res_a, res_b)
    kernel.last_logits = logits
    return out
